# revision 15
# baseline (speedup 1.0000x reference)
"""Trainium2 Bass kernel for nn_CustomRNNCell (Kuramoto-style RNN cell).

Strategy: pure data parallelism over the batch dim (B=512 -> 64 rows/core,
8 cores), parameters replicated.  All activations live on-chip in a
"transposed" layout [feature, batch] so every weight matrix is consumed by
the PE untransposed; the host does the input transposes / output
un-transposes / tensor packing (pure data movement).

Key algebraic restructurings (validated against the reference):
  * w_recover / b_recover are difference / strict-cumsum operators, so the
    basis-parameter prep is a square + shifted subtract + prefix scan on the
    DVE (no matmuls), and W63 = sum_i w_i is just the last column of wp^2.
  * delta_term = sin(d)*(F @ cos(d)) - cos(d)*(F @ sin(d))   (angle-difference
    expansion; kills the [B,A,A] sin grid)
  * piecewise-linear basis: with c = cumsum(b_t0^2) (knots, increasing),
      sum_i w_i*relu(f - c_i) = W63*relu(f) - sum_i w_i*min(c_i, relu(f))
    (the sum-w*c constants cancel between the two relu branches), computed
    with broadcast access patterns on the DVE; the fat min/mult/fold passes
    only ever see values <= c_max ~ 0.6 so they run in bf16, while the
    dominant W63*relu(f) term stays fp32.
  * clip(x,-m,m) = min(max(x,-m),m) as one tensor_scalar op.
  * new_state via one PSUM-accumulated matmul over the stacked
    [1; prev; inputs; delta_term; action] x [st3; st1; st3_Pm; st2; st4];
    everything not needing `action` accumulates while the basis runs.
  * params are host-packed into a handful of [128, N] panels -> one DMA each
    (the HWDGE ring serializes DMAs at ~0.6us apiece), split across the two
    HWDGE rings (sync + scalar).
"""

import sys

for _p in ("/opt/trn_rl_repo",):
    if _p not in sys.path:
        sys.path.insert(0, _p)

import numpy as np

import concourse.bacc as bacc
import concourse.mybir as mybir
import concourse.tile as tile
from concourse.bass_utils import run_bass_kernel_spmd
from concourse.masks import make_identity

B, A, I = 512, 256, 64
S, P = 512, 256
NCORES = 8
BL = B // NCORES  # 64 batch rows per core

DT = mybir.dt.float32
BF = mybir.dt.bfloat16
AX = mybir.AxisListType
ALU = mybir.AluOpType
ACTF = mybir.ActivationFunctionType

PI = float(np.pi)


def build_nc():
    nc = bacc.Bacc()

    # ---- DRAM I/O (host-packed panels) -------------------------------
    # pk: [128, (half, param, i)] = natural-layout wp_t0|bp_t0|wm_t0|bm_t0
    d_pk = nc.dram_tensor("pk", [128, 2 * 4 * I], DT, kind="ExternalInput")
    d_prevp = nc.dram_tensor("prevp", [128, 4 * BL], DT, kind="ExternalInput")
    d_selw = nc.dram_tensor("selwp", [128, 4 * A], DT, kind="ExternalInput")
    d_seld = nc.dram_tensor("seldp", [128, 4 * A], DT, kind="ExternalInput")
    d_sawma = nc.dram_tensor("sawma", [128, 5], DT, kind="ExternalInput")
    d_stFTp = nc.dram_tensor("stFTp", [128, 2 * A], DT, kind="ExternalInput")
    d_inpp = nc.dram_tensor("inpp", [128, 2 * BL], DT, kind="ExternalInput")
    d_st3 = nc.dram_tensor("st3", [1, S], DT, kind="ExternalInput")
    d_wnsa = nc.dram_tensor("wnsa", [128, 5 * S], DT, kind="ExternalInput")
    d_wnsb = nc.dram_tensor("wnsb", [128, 5 * S], DT, kind="ExternalInput")

    d_ns = nc.dram_tensor("ns_out", [BL, S], DT, kind="ExternalOutput")
    d_freq = nc.dram_tensor("freq_out", [BL, A], DT, kind="ExternalOutput")
    d_loss = nc.dram_tensor("loss_out", [BL, 1], DT, kind="ExternalOutput")
    d_actp = nc.dram_tensor("actp_out", [128, 2 * BL], DT, kind="ExternalOutput")

    with tile.TileContext(nc) as tc:
        with (
            tc.tile_pool(name="const", bufs=1) as cpool,
            tc.tile_pool(name="work", bufs=2) as wpool,
            tc.tile_pool(name="fat", bufs=3) as fatpool,
            tc.tile_pool(name="psum", bufs=6, space="PSUM") as ppool,
            tc.tile_pool(name="psum_ns", bufs=1, space="PSUM") as ppool_ns,
        ):
            # ---- input DMAs: one per panel, in order of need -----------
            selw = cpool.tile([128, 4 * A], DT, tag="selw")
            nc.sync.dma_start(out=selw[:], in_=d_selw[:])
            prevp = cpool.tile([128, 4 * BL], DT, tag="prevp")
            nc.sync.dma_start(out=prevp[:], in_=d_prevp[:])
            pk = cpool.tile([128, 2 * 4 * I], DT, tag="pk")
            nc.sync.dma_start(out=pk[:], in_=d_pk[:])
            wnsa = cpool.tile([128, 5 * S], DT, tag="wnsa")
            nc.sync.dma_start(out=wnsa[:], in_=d_wnsa[:])
            # the rest goes on the ACT HWDGE ring, in parallel
            sawma = cpool.tile([128, 5], DT, tag="sawma")
            nc.scalar.dma_start(out=sawma[:], in_=d_sawma[:])
            seld = cpool.tile([128, 4 * A], DT, tag="seld")
            nc.scalar.dma_start(out=seld[:], in_=d_seld[:])
            stFTp = cpool.tile([128, 2 * A], DT, tag="stFTp")
            nc.scalar.dma_start(out=stFTp[:], in_=d_stFTp[:])
            inpp = cpool.tile([128, 2 * BL], DT, tag="inpp")
            nc.scalar.dma_start(out=inpp[:], in_=d_inpp[:])
            st3 = cpool.tile([1, S], DT, tag="st3")
            nc.scalar.dma_start(out=st3[:], in_=d_st3[:])
            wnsb = cpool.tile([128, 5 * S], DT, tag="wnsb")
            nc.scalar.dma_start(out=wnsb[:], in_=d_wnsb[:])

            def prevT(k):
                return prevp[:, k * BL:(k + 1) * BL]

            def selwk(k):  # [128, 256] K-tile of select_w
                return selw[:, k * A:(k + 1) * A]

            def seldk(k):
                return seld[:, k * A:(k + 1) * A]

            def wns(k):
                if k < 5:
                    return wnsa[:, k * S:(k + 1) * S]
                return wnsb[:, (k - 5) * S:(k - 5 + 1) * S]

            ident = cpool.tile([BL, BL], DT, tag="ident")
            make_identity(nc, ident[:])
            ones_row = cpool.tile([1, BL], DT, tag="ones_row")
            nc.vector.memset(ones_row[:], 1.0)
            ma_col = sawma[:, 4:5]
            nma_col = cpool.tile([128, 1], DT, tag="nma")
            nc.vector.tensor_scalar(nma_col[:], ma_col, -1.0, None, ALU.mult)
            bias_hpi = cpool.tile([128, 1], DT, tag="bias_hpi")
            nc.vector.memset(bias_hpi[:], PI / 2)

            # ---- param prep, all on the DVE ------------------------------
            # sq2 = pk^2 ; per (half, param) slices are [128, 64]
            sq2 = wpool.tile([128, 2 * 4 * I], DT, tag="sq2")
            nc.vector.tensor_tensor(sq2[:], pk[:], pk[:], ALU.mult)

            def sqs(half, j):
                o = half * 4 * I + j * I
                return sq2[:, o:o + I], sq2[:, o + I - 1:o + I]  # slice, last col

            # w_plus = diff(wp2) (bf16), "wmb" = +diff(wm2) = -w_minus (bf16),
            # c = exclusive-cumsum(bp2), c' = exclusive-cumsum(bm2) (fp32)
            wpb, wmb, cK, cKm, W63p, W63m = [], [], [], [], [], []
            for half in range(2):
                wp2, wp2last = sqs(half, 0)
                bp2, _ = sqs(half, 1)
                wm2, wm2last = sqs(half, 2)
                bm2, _ = sqs(half, 3)
                W63p.append(wp2last)
                W63m.append(wm2last)  # positive; sign handled in finish_half

                t = wpool.tile([128, I], BF, tag=f"wpb{half}")
                nc.vector.tensor_copy(t[:, 0:1], wp2[:, 0:1])
                nc.vector.tensor_tensor(t[:, 1:I], wp2[:, 1:I], wp2[:, 0:I - 1],
                                        ALU.subtract)
                wpb.append(t)
                t = wpool.tile([128, I], BF, tag=f"wmb{half}")
                nc.vector.tensor_copy(t[:, 0:1], wm2[:, 0:1])
                nc.vector.tensor_tensor(t[:, 1:I], wm2[:, 1:I], wm2[:, 0:I - 1],
                                        ALU.subtract)
                wmb.append(t)
                for src2, lst in ((bp2, cK), (bm2, cKm)):
                    s = wpool.tile([128, I], DT, tag="scan")
                    nc.vector.tensor_tensor_scan(s[:], src2, src2, 0.0,
                                                 ALU.add, ALU.bypass)
                    cc = wpool.tile([128, I], DT, tag=f"c{len(lst)}_{half}")
                    nc.vector.tensor_tensor(cc[:], s[:], src2, ALU.subtract)
                    lst.append(cc)

            # ---- freq / delta:  fdT = sel^T-as-lhsT @ prevT --------------
            fd_ps = []
            for m in range(4):
                ps = ppool.tile([128, BL], DT, tag="ps")
                for k in range(4):
                    lhs = (selwk(k) if m < 2 else seldk(k))
                    mm = m % 2
                    nc.tensor.matmul(ps[:], lhs[:, mm * 128:(mm + 1) * 128],
                                     prevT(k), start=(k == 0), stop=(k == 3))
                fd_ps.append(ps)

            # r+ = relu(f), r- = relu(-f)  (fp32, straight from PSUM)
            r_p, r_m = [], []
            for half in range(2):
                rp = wpool.tile([128, BL], DT, tag=f"r_p{half}")
                nc.vector.tensor_scalar(rp[:], fd_ps[half][:], 0.0, None, ALU.max)
                rm = wpool.tile([128, BL], DT, tag=f"r_m{half}")
                nc.vector.tensor_scalar(rm[:], fd_ps[half][:], -1.0, 0.0,
                                        ALU.mult, ALU.max)
                r_p.append(rp)
                r_m.append(rm)

            # ---- basis fat-pass helper ----------------------------------
            def fat_unit(r, wb, cc, w63):
                r_bc = r[:].rearrange("p (b u) -> p b u", u=1).broadcast_to([128, BL, I])
                c_bc = cc[:].rearrange("p (u i) -> p u i", u=1).broadcast_to([128, BL, I])
                w_bc = wb[:].rearrange("p (u i) -> p u i", u=1).broadcast_to([128, BL, I])
                M = fatpool.tile([128, BL * I], BF, tag="M")
                M3 = M[:].rearrange("p (b i) -> p b i", i=I)
                nc.vector.tensor_tensor(M3, r_bc, c_bc, ALU.min)
                Y = fatpool.tile([128, BL * I], BF, tag="Y")
                Y3 = Y[:].rearrange("p (b i) -> p b i", i=I)
                nc.vector.tensor_tensor(Y3, M3, w_bc, ALU.mult)
                F1 = fatpool.tile([128, BL * 32], BF, tag="F1")
                F13 = F1[:].rearrange("p (b i) -> p b i", i=32)
                nc.vector.tensor_tensor(F13, Y3[:, :, 0:32], Y3[:, :, 32:64], ALU.add)
                F2 = fatpool.tile([128, BL * 16], BF, tag="F2")
                F23 = F2[:].rearrange("p (b i) -> p b i", i=16)
                nc.vector.tensor_tensor(F23, F13[:, :, 0:16], F13[:, :, 16:32], ALU.add)
                red = wpool.tile([128, BL], DT, tag="red")
                nc.vector.tensor_reduce(red[:], F23, AX.X, ALU.add)
                t = wpool.tile([128, BL], DT, tag="tbr")
                nc.vector.scalar_tensor_tensor(t[:], r[:], w63, red[:],
                                               ALU.mult, ALU.subtract)
                return t

            actp = wpool.tile([128, 2 * BL], DT, tag="actp")
            actT = []

            def finish_half(half, tp, tm):
                # anc = tp - tm  (tm carries +diff weights = -w_minus terms)
                anc = wpool.tile([128, BL], DT, tag="anc")
                nc.vector.tensor_tensor(anc[:], tp[:], tm[:], ALU.subtract)
                at = actp[:, half * BL:(half + 1) * BL]
                nc.vector.tensor_scalar(at, anc[:], nma_col[:], ma_col,
                                        ALU.max, ALU.min)
                actT.append(at)

            # FAT unit (+,0) goes first so the DVE gets busy asap
            t_p0 = fat_unit(r_p[0], wpb[0], cK[0], W63p[0])

            # ---- sin / cos of delta (range-reduced; fills DVE/ACT gaps) --
            # y = x - 2pi*k via an int32 cast (round-to-nearest on HW,
            # trunc in CoreSim); a branch-free +-2pi correction makes the
            # result [-pi, pi] under either conversion mode.
            sinT, cosT = [], []
            for half in range(2):
                ki = wpool.tile([128, BL], mybir.dt.int32, tag="sc_ki")
                nc.vector.tensor_scalar(ki[:], fd_ps[2 + half][:],
                                        float(1 / (2 * PI)), 32.0, ALU.mult, ALU.add)
                xoff = wpool.tile([128, BL], DT, tag="sc_xoff")
                nc.vector.tensor_scalar(xoff[:], fd_ps[2 + half][:], float(64 * PI),
                                        None, ALU.add)
                y1 = wpool.tile([128, BL], DT, tag="sc_y1")
                nc.vector.scalar_tensor_tensor(y1[:], ki[:], float(-2 * PI), xoff[:],
                                               ALU.mult, ALU.add)
                w = wpool.tile([128, BL], DT, tag="sc_w")
                nc.vector.tensor_scalar(w[:], y1[:], PI, float(-2 * PI),
                                        ALU.is_gt, ALU.mult)
                y2 = wpool.tile([128, BL], DT, tag="sc_y2")
                nc.vector.tensor_tensor(y2[:], y1[:], w[:], ALU.add)
                y = wpool.tile([128, BL], DT, tag="sc_y")
                nc.vector.tensor_scalar(y[:], y2[:], -PI, PI, ALU.max, ALU.min)
                s = wpool.tile([128, BL], DT, tag=f"sinT{half}")
                nc.scalar.activation(s[:], y[:], ACTF.Sin)
                # cos(y) = sin(pi/2 - |y|),  argument stays in [-pi/2, pi/2]
                ay = wpool.tile([128, BL], DT, tag="sc_ay")
                nc.scalar.activation(ay[:], y[:], ACTF.Abs)
                c = wpool.tile([128, BL], DT, tag=f"cosT{half}")
                nc.scalar.activation(c[:], ay[:], ACTF.Sin, bias=bias_hpi[:],
                                     scale=-1.0)
                sinT.append(s)
                cosT.append(c)

            t_m0 = fat_unit(r_m[0], wmb[0], cKm[0], W63m[0])
            finish_half(0, t_p0, t_m0)

            # ---- U = F @ cos, V = F @ sin ; dtT = sin*U - cos*V ----------
            dtT = []
            for m in range(2):
                psU = ppool.tile([128, BL], DT, tag="ps")
                psV = ppool.tile([128, BL], DT, tag="ps")
                for k in range(2):
                    lhs = stFTp[:, k * A + m * 128:k * A + (m + 1) * 128]
                    nc.tensor.matmul(psU[:], lhs, cosT[k][:], start=(k == 0), stop=(k == 1))
                    uv_last = nc.tensor.matmul(psV[:], lhs, sinT[k][:],
                                               start=(k == 0), stop=(k == 1))
                t1 = wpool.tile([128, BL], DT, tag="dt_t1")
                nc.vector.tensor_tensor(t1[:], sinT[m][:], psU[:], ALU.mult)
                t2 = wpool.tile([128, BL], DT, tag="dt_t2")
                nc.vector.tensor_tensor(t2[:], cosT[m][:], psV[:], ALU.mult)
                t = wpool.tile([128, BL], DT, tag=f"dtT{m}")
                nc.vector.tensor_tensor(t[:], t1[:], t2[:], ALU.subtract)
                dtT.append(t)

            # ---- new_state stacked matmul: everything that doesn't need
            # action accumulates into PSUM while the basis runs -------------
            ns_ps = ppool_ns.tile([BL, S], DT, tag="ns_ps")
            ns_first = nc.tensor.matmul(ns_ps[:], ones_row[:], st3[:],
                                        start=True, stop=False)
            tile.add_dep_helper(ns_first.ins, uv_last.ins, sync=False,
                                reason="keep PE free for U/V before the wns stack")
            for k in range(4):
                nc.tensor.matmul(ns_ps[:], prevT(k), wns(k), start=False, stop=False)
            for k in range(2):
                nc.tensor.matmul(ns_ps[:], inpp[:, k * BL:(k + 1) * BL], wns(6 + k),
                                 start=False, stop=False)
            for k in range(2):
                nc.tensor.matmul(ns_ps[:], dtT[k][:], wns(4 + k),
                                 start=False, stop=False)
            # action half 0 as soon as it exists (wns block 8 = st4 rows 0:128)
            nc.tensor.matmul(ns_ps[:], actT[0], wns(8), start=False, stop=False)

            # remaining fat units
            t_p1 = fat_unit(r_p[1], wpb[1], cK[1], W63p[1])
            t_m1 = fat_unit(r_m[1], wmb[1], cKm[1], W63m[1])
            finish_half(1, t_p1, t_m1)
            nc.sync.dma_start(out=d_actp[:], in_=actp[:])
            nc.tensor.matmul(ns_ps[:], actT[1], wns(9), start=False, stop=True)

            ns_nat = wpool.tile([BL, S], DT, tag="ns_nat")
            nc.vector.tensor_copy(ns_nat[:], ns_ps[:])
            nc.sync.dma_start(out=d_ns[:], in_=ns_nat[:])

            # ---- transpose new_state back to [s, b] for the out matmuls --
            nsT, ns2T = [], []
            for k in range(4):
                ps = ppool.tile([128, BL], DT, tag="ps")
                nc.tensor.transpose(ps[:], ns_nat[:, k * 128:(k + 1) * 128], ident[:])
                t = wpool.tile([128, BL], DT, tag=f"nsT{k}")
                nc.vector.tensor_copy(t[:], ps[:])
                nsT.append(t)
                t2 = wpool.tile([128, BL], DT, tag=f"ns2T{k}")
                nc.vector.tensor_tensor(t2[:], t[:], t[:], ALU.mult)
                ns2T.append(t2)

            # ---- frequency = ns @ select_w ; loss0 = ns^2 @ saw ----------
            fq_ps = ppool.tile([BL, A], DT, tag="ps")
            for k in range(4):
                nc.tensor.matmul(fq_ps[:], nsT[k][:], selwk(k),
                                 start=(k == 0), stop=(k == 3))
            fq = wpool.tile([BL, A], DT, tag="fq")
            nc.vector.tensor_copy(fq[:], fq_ps[:])
            nc.sync.dma_start(out=d_freq[:], in_=fq[:])

            ls_ps = ppool.tile([BL, 1], DT, tag="ps")
            for k in range(4):
                nc.tensor.matmul(ls_ps[:], ns2T[k][:], sawma[:, k:k + 1],
                                 start=(k == 0), stop=(k == 3))
            ls = wpool.tile([BL, 1], DT, tag="ls")
            nc.vector.tensor_copy(ls[:], ls_ps[:])
            nc.sync.dma_start(out=d_loss[:], in_=ls[:])

    nc.compile()
    return nc


_NC_CACHE = None


def _get_nc():
    global _NC_CACHE
    if _NC_CACHE is None:
        _NC_CACHE = build_nc()
    return _NC_CACHE


def _pack_rows(a, width):
    """[R, C] with R = 128*n  ->  [128, n*C] panel (blocks along free dim)."""
    r, c = a.shape
    n = r // 128
    return np.ascontiguousarray(
        a.reshape(n, 128, c).transpose(1, 0, 2).reshape(128, n * c))


def prepare_in_maps(inputs):
    x = {k: np.asarray(v) for k, v in inputs.items()}
    f32 = lambda a: np.ascontiguousarray(a, dtype=np.float32)

    wns = np.concatenate([x["state_transfer1"], x["state_transfer2"],
                          x["state_transfer3_Pm"], x["state_transfer4"]], axis=0)
    sawma = np.zeros((128, 5), np.float32)
    sawma[:, 0:4] = x["select_add_w"].reshape(4, 128).T
    sawma[:, 4] = np.float32(x["max_action"])

    shared = {
        "pk": f32(_pack_rows(np.concatenate(
            [x["w_plus_temp0"], x["b_plus_temp0"],
             x["w_minus_temp0"], x["b_minus_temp0"]], axis=1), 4 * I)),
        "selwp": f32(_pack_rows(x["select_w"], A)),
        "seldp": f32(_pack_rows(x["select_delta"], A)),
        "sawma": sawma,
        "stFTp": f32(_pack_rows(x["state_transferF"].T, A)),
        "st3": f32(x["state_transfer3"][None, :]),
        "wnsa": f32(_pack_rows(wns[:640], S)),
        "wnsb": f32(_pack_rows(wns[640:], S)),
    }
    in_maps = []
    for k in range(NCORES):
        sl = slice(k * BL, (k + 1) * BL)
        m = dict(shared)
        m["prevp"] = f32(_pack_rows(x["prev_output"][sl].T, BL))
        m["inpp"] = f32(_pack_rows(x["inputs"][sl].T, BL))
        in_maps.append(m)
    return in_maps


def postprocess(res):
    loss0 = np.concatenate([res[k]["loss_out"] for k in range(NCORES)], axis=0)
    frequency = np.concatenate([res[k]["freq_out"] for k in range(NCORES)], axis=0)
    action = np.concatenate(
        [np.concatenate([res[k]["actp_out"][:, 0:BL],
                         res[k]["actp_out"][:, BL:2 * BL]], axis=0).T
         for k in range(NCORES)], axis=0)
    new_state = np.concatenate([res[k]["ns_out"] for k in range(NCORES)], axis=0)
    return (loss0, frequency, action, new_state)


def kernel(**inputs):
    in_maps = prepare_in_maps(inputs)
    res = run_bass_kernel_spmd(_get_nc(), in_maps, list(range(NCORES))).results
    return postprocess(res)


# revision 16
# speedup vs baseline: 1.0818x; 1.0818x over previous
"""Trainium2 Bass kernel for nn_CustomRNNCell (Kuramoto-style RNN cell).

Strategy: pure data parallelism over the batch dim (B=512 -> 64 rows/core,
8 cores), parameters replicated.  All activations live on-chip in a
"transposed" layout [feature, batch] so every weight matrix is consumed by
the PE untransposed; the host does the input transposes / output
un-transposes / tensor packing (pure data movement).

Key algebraic restructurings (validated against the reference):
  * w_recover / b_recover are difference / strict-cumsum operators, so the
    basis-parameter prep is a square + shifted subtract + prefix scan on the
    DVE (no matmuls), and W63 = sum_i w_i is just the last column of wp^2.
  * delta_term = sin(d)*(F @ cos(d)) - cos(d)*(F @ sin(d))   (angle-difference
    expansion; kills the [B,A,A] sin grid)
  * piecewise-linear basis: with c = cumsum(b_t0^2) (knots, increasing),
      sum_i w_i*relu(f - c_i) = W63*relu(f) - sum_i w_i*min(c_i, relu(f))
    (the sum-w*c constants cancel between the two relu branches), computed
    with broadcast access patterns on the DVE; the fat min/mult/fold passes
    only ever see values <= c_max ~ 0.6 so they run in bf16, while the
    dominant W63*relu(f) term stays fp32.
  * clip(x,-m,m) = min(max(x,-m),m) as one tensor_scalar op.
  * new_state via one PSUM-accumulated matmul over the stacked
    [1; prev; inputs; delta_term; action] x [st3; st1; st3_Pm; st2; st4];
    everything not needing `action` accumulates while the basis runs.
  * params are host-packed into a handful of [128, N] panels -> one DMA each
    (the HWDGE ring serializes DMAs at ~0.6us apiece), split across the two
    HWDGE rings (sync + scalar).
"""

import sys

for _p in ("/opt/trn_rl_repo",):
    if _p not in sys.path:
        sys.path.insert(0, _p)

import numpy as np

import concourse.bacc as bacc
import concourse.mybir as mybir
import concourse.tile as tile
from concourse.bass_utils import run_bass_kernel_spmd
from concourse.masks import make_identity

B, A, I = 512, 256, 64
S, P = 512, 256
NCORES = 8
BL = B // NCORES  # 64 batch rows per core

DT = mybir.dt.float32
BF = mybir.dt.bfloat16
AX = mybir.AxisListType
ALU = mybir.AluOpType
ACTF = mybir.ActivationFunctionType

PI = float(np.pi)


def build_nc():
    nc = bacc.Bacc()

    # ---- DRAM I/O (host-packed panels) -------------------------------
    # pk: [128, (half, param, i)] = natural-layout wp_t0|bp_t0|wm_t0|bm_t0
    d_pk = nc.dram_tensor("pk", [128, 2 * 4 * I], DT, kind="ExternalInput")
    d_prevp = nc.dram_tensor("prevp", [128, 4 * BL], DT, kind="ExternalInput")
    d_selw = nc.dram_tensor("selwp", [128, 4 * A], DT, kind="ExternalInput")
    d_seld = nc.dram_tensor("seldp", [128, 4 * A], DT, kind="ExternalInput")
    d_sawma = nc.dram_tensor("sawma", [128, 5], DT, kind="ExternalInput")
    d_stFTp = nc.dram_tensor("stFTp", [128, 2 * A], DT, kind="ExternalInput")
    d_inpp = nc.dram_tensor("inpp", [128, 2 * BL], DT, kind="ExternalInput")
    d_st3 = nc.dram_tensor("st3", [1, S], DT, kind="ExternalInput")
    d_wnsa = nc.dram_tensor("wnsa", [128, 5 * S], DT, kind="ExternalInput")
    d_wnsb = nc.dram_tensor("wnsb", [128, 5 * S], DT, kind="ExternalInput")

    d_ns = nc.dram_tensor("ns_out", [BL, S], DT, kind="ExternalOutput")
    d_fql = nc.dram_tensor("fql_out", [BL, A + 1], DT, kind="ExternalOutput")
    d_actp = nc.dram_tensor("actp_out", [128, 2 * BL], DT, kind="ExternalOutput")

    with tile.TileContext(nc) as tc:
        with (
            tc.tile_pool(name="const", bufs=1) as cpool,
            tc.tile_pool(name="work", bufs=2) as wpool,
            tc.tile_pool(name="fat", bufs=3) as fatpool,
            tc.tile_pool(name="psum", bufs=6, space="PSUM") as ppool,
            tc.tile_pool(name="psum_ns", bufs=1, space="PSUM") as ppool_ns,
        ):
            # ---- input DMAs: one per panel, in order of need -----------
            selw = cpool.tile([128, 4 * A], DT, tag="selw")
            nc.sync.dma_start(out=selw[:], in_=d_selw[:])
            prevp = cpool.tile([128, 4 * BL], DT, tag="prevp")
            nc.sync.dma_start(out=prevp[:], in_=d_prevp[:])
            pk = cpool.tile([128, 2 * 4 * I], DT, tag="pk")
            nc.sync.dma_start(out=pk[:], in_=d_pk[:])
            # the rest goes on the ACT HWDGE ring, in parallel
            sawma = cpool.tile([128, 5], DT, tag="sawma")
            nc.scalar.dma_start(out=sawma[:], in_=d_sawma[:])
            seld = cpool.tile([128, 4 * A], DT, tag="seld")
            nc.scalar.dma_start(out=seld[:], in_=d_seld[:])
            stFTp = cpool.tile([128, 2 * A], DT, tag="stFTp")
            nc.scalar.dma_start(out=stFTp[:], in_=d_stFTp[:])
            inpp = cpool.tile([128, 2 * BL], DT, tag="inpp")
            nc.scalar.dma_start(out=inpp[:], in_=d_inpp[:])
            st3 = cpool.tile([1, S], DT, tag="st3")
            nc.scalar.dma_start(out=st3[:], in_=d_st3[:])
            wnsa = cpool.tile([128, 5 * S], DT, tag="wnsa")
            nc.scalar.dma_start(out=wnsa[:], in_=d_wnsa[:])
            wnsb = cpool.tile([128, 5 * S], DT, tag="wnsb")
            nc.scalar.dma_start(out=wnsb[:], in_=d_wnsb[:])

            def prevT(k):
                return prevp[:, k * BL:(k + 1) * BL]

            def selwk(k):  # [128, 256] K-tile of select_w
                return selw[:, k * A:(k + 1) * A]

            def seldk(k):
                return seld[:, k * A:(k + 1) * A]

            def wns(k):
                if k < 5:
                    return wnsa[:, k * S:(k + 1) * S]
                return wnsb[:, (k - 5) * S:(k - 5 + 1) * S]

            ident = cpool.tile([BL, BL], DT, tag="ident")
            make_identity(nc, ident[:])
            ones_row = cpool.tile([1, BL], DT, tag="ones_row")
            nc.vector.memset(ones_row[:], 1.0)
            ma_col = sawma[:, 4:5]
            nma_col = cpool.tile([128, 1], DT, tag="nma")
            nc.vector.tensor_scalar(nma_col[:], ma_col, -1.0, None, ALU.mult)
            bias_hpi = cpool.tile([128, 1], DT, tag="bias_hpi")
            nc.vector.memset(bias_hpi[:], PI / 2)

            # ---- param prep, all on the DVE ------------------------------
            # sq2 = pk^2 ; per (half, param) slices are [128, 64]
            sq2 = wpool.tile([128, 2 * 4 * I], DT, tag="sq2")
            nc.vector.tensor_tensor(sq2[:], pk[:], pk[:], ALU.mult)

            def sqs(half, j):
                o = half * 4 * I + j * I
                return sq2[:, o:o + I], sq2[:, o + I - 1:o + I]  # slice, last col

            # w_plus = diff(wp2) (bf16), "wmb" = +diff(wm2) = -w_minus (bf16),
            # c = exclusive-cumsum(bp2), c' = exclusive-cumsum(bm2) (fp32)
            wpb, wmb, cK, cKm, W63p, W63m = [], [], [], [], [], []
            for half in range(2):
                wp2, wp2last = sqs(half, 0)
                bp2, _ = sqs(half, 1)
                wm2, wm2last = sqs(half, 2)
                bm2, _ = sqs(half, 3)
                W63p.append(wp2last)
                W63m.append(wm2last)  # positive; sign handled in finish_half

                t = wpool.tile([128, I], BF, tag=f"wpb{half}")
                nc.vector.tensor_copy(t[:, 0:1], wp2[:, 0:1])
                nc.vector.tensor_tensor(t[:, 1:I], wp2[:, 1:I], wp2[:, 0:I - 1],
                                        ALU.subtract)
                wpb.append(t)
                t = wpool.tile([128, I], BF, tag=f"wmb{half}")
                nc.vector.tensor_copy(t[:, 0:1], wm2[:, 0:1])
                nc.vector.tensor_tensor(t[:, 1:I], wm2[:, 1:I], wm2[:, 0:I - 1],
                                        ALU.subtract)
                wmb.append(t)
                for src2, lst in ((bp2, cK), (bm2, cKm)):
                    s = wpool.tile([128, I], DT, tag="scan")
                    nc.vector.tensor_tensor_scan(s[:], src2, src2, 0.0,
                                                 ALU.add, ALU.bypass)
                    cc = wpool.tile([128, I], DT, tag=f"c{len(lst)}_{half}")
                    nc.vector.tensor_tensor(cc[:], s[:], src2, ALU.subtract)
                    lst.append(cc)

            # ---- freq / delta:  fdT = sel^T-as-lhsT @ prevT --------------
            fd_ps = []
            for m in range(4):
                ps = ppool.tile([128, BL], DT, tag="ps")
                for k in range(4):
                    lhs = (selwk(k) if m < 2 else seldk(k))
                    mm = m % 2
                    nc.tensor.matmul(ps[:], lhs[:, mm * 128:(mm + 1) * 128],
                                     prevT(k), start=(k == 0), stop=(k == 3))
                fd_ps.append(ps)

            # r+ = relu(f), r- = relu(-f)  (fp32, straight from PSUM)
            r_p, r_m = [], []
            for half in range(2):
                rp = wpool.tile([128, BL], DT, tag=f"r_p{half}")
                nc.vector.tensor_scalar(rp[:], fd_ps[half][:], 0.0, None, ALU.max)
                rm = wpool.tile([128, BL], DT, tag=f"r_m{half}")
                nc.vector.tensor_scalar(rm[:], fd_ps[half][:], -1.0, 0.0,
                                        ALU.mult, ALU.max)
                r_p.append(rp)
                r_m.append(rm)

            # ---- basis fat-pass helper ----------------------------------
            def fat_unit(r, wb, cc, w63):
                r_bc = r[:].rearrange("p (b u) -> p b u", u=1).broadcast_to([128, BL, I])
                c_bc = cc[:].rearrange("p (u i) -> p u i", u=1).broadcast_to([128, BL, I])
                w_bc = wb[:].rearrange("p (u i) -> p u i", u=1).broadcast_to([128, BL, I])
                M = fatpool.tile([128, BL * I], BF, tag="M")
                M3 = M[:].rearrange("p (b i) -> p b i", i=I)
                nc.vector.tensor_tensor(M3, r_bc, c_bc, ALU.min)
                Y = fatpool.tile([128, BL * I], BF, tag="Y")
                Y3 = Y[:].rearrange("p (b i) -> p b i", i=I)
                nc.vector.tensor_tensor(Y3, M3, w_bc, ALU.mult)
                F1 = fatpool.tile([128, BL * 32], BF, tag="F1")
                F13 = F1[:].rearrange("p (b i) -> p b i", i=32)
                nc.vector.tensor_tensor(F13, Y3[:, :, 0:32], Y3[:, :, 32:64], ALU.add)
                F2 = fatpool.tile([128, BL * 16], BF, tag="F2")
                F23 = F2[:].rearrange("p (b i) -> p b i", i=16)
                nc.vector.tensor_tensor(F23, F13[:, :, 0:16], F13[:, :, 16:32], ALU.add)
                red = wpool.tile([128, BL], DT, tag="red")
                nc.vector.tensor_reduce(red[:], F23, AX.X, ALU.add)
                t = wpool.tile([128, BL], DT, tag="tbr")
                nc.vector.scalar_tensor_tensor(t[:], r[:], w63, red[:],
                                               ALU.mult, ALU.subtract)
                return t

            actp = wpool.tile([128, 2 * BL], DT, tag="actp")
            actT = []

            def finish_half(half, tp, tm):
                # anc = tp - tm  (tm carries +diff weights = -w_minus terms)
                anc = wpool.tile([128, BL], DT, tag="anc")
                nc.vector.tensor_tensor(anc[:], tp[:], tm[:], ALU.subtract)
                at = actp[:, half * BL:(half + 1) * BL]
                nc.vector.tensor_scalar(at, anc[:], nma_col[:], ma_col,
                                        ALU.max, ALU.min)
                actT.append(at)

            # FAT unit (+,0) goes first so the DVE gets busy asap
            t_p0 = fat_unit(r_p[0], wpb[0], cK[0], W63p[0])

            # ---- sin / cos of delta (range-reduced; fills DVE/ACT gaps) --
            # y = x - 2pi*k via an int32 cast (round-to-nearest on HW,
            # trunc in CoreSim); a branch-free +-2pi correction makes the
            # result [-pi, pi] under either conversion mode.
            sinT, cosT = [], []
            for half in range(2):
                ki = wpool.tile([128, BL], mybir.dt.int32, tag="sc_ki")
                nc.vector.tensor_scalar(ki[:], fd_ps[2 + half][:],
                                        float(1 / (2 * PI)), 32.0, ALU.mult, ALU.add)
                xoff = wpool.tile([128, BL], DT, tag="sc_xoff")
                nc.vector.tensor_scalar(xoff[:], fd_ps[2 + half][:], float(64 * PI),
                                        None, ALU.add)
                y1 = wpool.tile([128, BL], DT, tag="sc_y1")
                nc.vector.scalar_tensor_tensor(y1[:], ki[:], float(-2 * PI), xoff[:],
                                               ALU.mult, ALU.add)
                w = wpool.tile([128, BL], DT, tag="sc_w")
                nc.vector.tensor_scalar(w[:], y1[:], PI, float(-2 * PI),
                                        ALU.is_gt, ALU.mult)
                y2 = wpool.tile([128, BL], DT, tag="sc_y2")
                nc.vector.tensor_tensor(y2[:], y1[:], w[:], ALU.add)
                y = wpool.tile([128, BL], DT, tag="sc_y")
                nc.vector.tensor_scalar(y[:], y2[:], -PI, PI, ALU.max, ALU.min)
                s = wpool.tile([128, BL], DT, tag=f"sinT{half}")
                nc.scalar.activation(s[:], y[:], ACTF.Sin)
                # cos(y) = sin(pi/2 - |y|),  argument stays in [-pi/2, pi/2]
                ay = wpool.tile([128, BL], DT, tag="sc_ay")
                nc.scalar.activation(ay[:], y[:], ACTF.Abs)
                c = wpool.tile([128, BL], DT, tag=f"cosT{half}")
                nc.scalar.activation(c[:], ay[:], ACTF.Sin, bias=bias_hpi[:],
                                     scale=-1.0)
                sinT.append(s)
                cosT.append(c)

            t_m0 = fat_unit(r_m[0], wmb[0], cKm[0], W63m[0])
            finish_half(0, t_p0, t_m0)

            # ---- U = F @ cos, V = F @ sin ; dtT = sin*U - cos*V ----------
            dtT = []
            for m in range(2):
                psU = ppool.tile([128, BL], DT, tag="ps")
                psV = ppool.tile([128, BL], DT, tag="ps")
                for k in range(2):
                    lhs = stFTp[:, k * A + m * 128:k * A + (m + 1) * 128]
                    nc.tensor.matmul(psU[:], lhs, cosT[k][:], start=(k == 0), stop=(k == 1))
                    uv_last = nc.tensor.matmul(psV[:], lhs, sinT[k][:],
                                               start=(k == 0), stop=(k == 1))
                t1 = wpool.tile([128, BL], DT, tag="dt_t1")
                nc.vector.tensor_tensor(t1[:], sinT[m][:], psU[:], ALU.mult)
                t2 = wpool.tile([128, BL], DT, tag="dt_t2")
                nc.vector.tensor_tensor(t2[:], cosT[m][:], psV[:], ALU.mult)
                t = wpool.tile([128, BL], DT, tag=f"dtT{m}")
                nc.vector.tensor_tensor(t[:], t1[:], t2[:], ALU.subtract)
                dtT.append(t)

            # ---- new_state stacked matmul: everything that doesn't need
            # action accumulates into PSUM while the basis runs -------------
            ns_ps = ppool_ns.tile([BL, S], DT, tag="ns_ps")
            ns_first = nc.tensor.matmul(ns_ps[:], ones_row[:], st3[:],
                                        start=True, stop=False)
            tile.add_dep_helper(ns_first.ins, uv_last.ins, sync=False,
                                reason="keep PE free for U/V before the wns stack")
            for k in range(4):
                nc.tensor.matmul(ns_ps[:], prevT(k), wns(k), start=False, stop=False)
            for k in range(2):
                nc.tensor.matmul(ns_ps[:], inpp[:, k * BL:(k + 1) * BL], wns(6 + k),
                                 start=False, stop=False)
            for k in range(2):
                nc.tensor.matmul(ns_ps[:], dtT[k][:], wns(4 + k),
                                 start=False, stop=False)
            # action half 0 as soon as it exists (wns block 8 = st4 rows 0:128)
            nc.tensor.matmul(ns_ps[:], actT[0], wns(8), start=False, stop=False)

            # remaining fat units
            t_p1 = fat_unit(r_p[1], wpb[1], cK[1], W63p[1])
            t_m1 = fat_unit(r_m[1], wmb[1], cKm[1], W63m[1])
            finish_half(1, t_p1, t_m1)
            nc.sync.dma_start(out=d_actp[:], in_=actp[:])
            nc.tensor.matmul(ns_ps[:], actT[1], wns(9), start=False, stop=True)

            ns_nat = wpool.tile([BL, S], DT, tag="ns_nat")
            nc.vector.tensor_copy(ns_nat[:], ns_ps[:])
            nc.sync.dma_start(out=d_ns[:], in_=ns_nat[:])

            # ---- transpose new_state back to [s, b] for the out matmuls --
            nsT, ns2T = [], []
            for k in range(4):
                ps = ppool.tile([128, BL], DT, tag="ps")
                nc.tensor.transpose(ps[:], ns_nat[:, k * 128:(k + 1) * 128], ident[:])
                t = wpool.tile([128, BL], DT, tag=f"nsT{k}")
                nc.vector.tensor_copy(t[:], ps[:])
                nsT.append(t)
                t2 = wpool.tile([128, BL], DT, tag=f"ns2T{k}")
                nc.vector.tensor_tensor(t2[:], t[:], t[:], ALU.mult)
                ns2T.append(t2)

            # ---- frequency = ns @ select_w ; loss0 = ns^2 @ saw ----------
            fq_ps = ppool.tile([BL, A], DT, tag="ps")
            for k in range(4):
                nc.tensor.matmul(fq_ps[:], nsT[k][:], selwk(k),
                                 start=(k == 0), stop=(k == 3))
            fql = wpool.tile([BL, A + 1], DT, tag="fql")
            nc.vector.tensor_copy(fql[:, 0:A], fq_ps[:])

            ls_ps = ppool.tile([BL, 1], DT, tag="ps")
            for k in range(4):
                nc.tensor.matmul(ls_ps[:], ns2T[k][:], sawma[:, k:k + 1],
                                 start=(k == 0), stop=(k == 3))
            nc.vector.tensor_copy(fql[:, A:A + 1], ls_ps[:])
            nc.sync.dma_start(out=d_fql[:], in_=fql[:])

    nc.compile()
    return nc


_NC_CACHE = None


def _get_nc():
    global _NC_CACHE
    if _NC_CACHE is None:
        _NC_CACHE = build_nc()
    return _NC_CACHE


def _pack_rows(a, width):
    """[R, C] with R = 128*n  ->  [128, n*C] panel (blocks along free dim)."""
    r, c = a.shape
    n = r // 128
    return np.ascontiguousarray(
        a.reshape(n, 128, c).transpose(1, 0, 2).reshape(128, n * c))


def prepare_in_maps(inputs):
    x = {k: np.asarray(v) for k, v in inputs.items()}
    f32 = lambda a: np.ascontiguousarray(a, dtype=np.float32)

    wns = np.concatenate([x["state_transfer1"], x["state_transfer2"],
                          x["state_transfer3_Pm"], x["state_transfer4"]], axis=0)
    sawma = np.zeros((128, 5), np.float32)
    sawma[:, 0:4] = x["select_add_w"].reshape(4, 128).T
    sawma[:, 4] = np.float32(x["max_action"])

    shared = {
        "pk": f32(_pack_rows(np.concatenate(
            [x["w_plus_temp0"], x["b_plus_temp0"],
             x["w_minus_temp0"], x["b_minus_temp0"]], axis=1), 4 * I)),
        "selwp": f32(_pack_rows(x["select_w"], A)),
        "seldp": f32(_pack_rows(x["select_delta"], A)),
        "sawma": sawma,
        "stFTp": f32(_pack_rows(x["state_transferF"].T, A)),
        "st3": f32(x["state_transfer3"][None, :]),
        "wnsa": f32(_pack_rows(wns[:640], S)),
        "wnsb": f32(_pack_rows(wns[640:], S)),
    }
    in_maps = []
    for k in range(NCORES):
        sl = slice(k * BL, (k + 1) * BL)
        m = dict(shared)
        m["prevp"] = f32(_pack_rows(x["prev_output"][sl].T, BL))
        m["inpp"] = f32(_pack_rows(x["inputs"][sl].T, BL))
        in_maps.append(m)
    return in_maps


def postprocess(res):
    loss0 = np.concatenate([res[k]["fql_out"][:, A:A + 1] for k in range(NCORES)], axis=0)
    frequency = np.concatenate([res[k]["fql_out"][:, 0:A] for k in range(NCORES)], axis=0)
    action = np.concatenate(
        [np.concatenate([res[k]["actp_out"][:, 0:BL],
                         res[k]["actp_out"][:, BL:2 * BL]], axis=0).T
         for k in range(NCORES)], axis=0)
    new_state = np.concatenate([res[k]["ns_out"] for k in range(NCORES)], axis=0)
    return (loss0, frequency, action, new_state)


def kernel(**inputs):
    in_maps = prepare_in_maps(inputs)
    res = run_bass_kernel_spmd(_get_nc(), in_maps, list(range(NCORES))).results
    return postprocess(res)


# revision 19
# speedup vs baseline: 1.2343x; 1.1411x over previous
"""Trainium2 Bass kernel for nn_CustomRNNCell (Kuramoto-style RNN cell).

Strategy: pure data parallelism over the batch dim (B=512 -> 64 rows/core,
8 cores), parameters replicated.  All activations live on-chip in a
"transposed" layout [feature, batch] so every weight matrix is consumed by
the PE untransposed; the host does the input transposes / output
un-transposes / tensor packing (pure data movement).

Key algebraic restructurings (validated against the reference):
  * w_recover / b_recover are difference / strict-cumsum operators, so the
    basis-parameter prep is a square + shifted subtract + prefix scan on the
    DVE (no matmuls), and W63 = sum_i w_i is just the last column of wp^2.
  * delta_term = sin(d)*(F @ cos(d)) - cos(d)*(F @ sin(d))   (angle-difference
    expansion; kills the [B,A,A] sin grid)
  * piecewise-linear basis: with c = cumsum(b_t0^2) (knots, increasing),
      sum_i w_i*relu(f - c_i) = W63*relu(f) - sum_i w_i*min(c_i, relu(f))
    (the sum-w*c constants cancel between the two relu branches), computed
    with broadcast access patterns on the DVE; the fat min/mult/fold passes
    only ever see values <= c_max ~ 0.6 so they run in bf16, while the
    dominant W63*relu(f) term stays fp32.
  * clip(x,-m,m) = min(max(x,-m),m) as one tensor_scalar op.
  * new_state via one PSUM-accumulated matmul over the stacked
    [1; prev; inputs; delta_term; action] x [st3; st1; st3_Pm; st2; st4];
    everything not needing `action` accumulates while the basis runs.
  * params are host-packed into a handful of [128, N] panels -> one DMA each
    (the HWDGE ring serializes DMAs at ~0.6us apiece), split across the two
    HWDGE rings (sync + scalar).
"""

import sys

for _p in ("/opt/trn_rl_repo",):
    if _p not in sys.path:
        sys.path.insert(0, _p)

import numpy as np

import concourse.bacc as bacc
import concourse.mybir as mybir
import concourse.tile as tile
from concourse.bass_utils import run_bass_kernel_spmd
from concourse.masks import make_identity

B, A, I = 512, 256, 64
S, P = 512, 256
NCORES = 8
BL = B // NCORES  # 64 batch rows per core

DT = mybir.dt.float32
BF = mybir.dt.bfloat16
AX = mybir.AxisListType
ALU = mybir.AluOpType
ACTF = mybir.ActivationFunctionType

PI = float(np.pi)


def build_nc():
    nc = bacc.Bacc()

    # ---- DRAM I/O (host-packed panels) -------------------------------
    # pk: [128, (half, param, i)] = natural-layout wp_t0|bp_t0|wm_t0|bm_t0
    d_pk = nc.dram_tensor("pk", [128, 2 * 4 * I], DT, kind="ExternalInput")
    d_prevp = nc.dram_tensor("prevp", [128, 4 * BL], DT, kind="ExternalInput")
    d_selw = nc.dram_tensor("selwp", [128, 4 * A], DT, kind="ExternalInput")
    d_seld = nc.dram_tensor("seldp", [128, 4 * A], DT, kind="ExternalInput")
    d_sawma = nc.dram_tensor("sawma", [128, 5], DT, kind="ExternalInput")
    d_stFTp = nc.dram_tensor("stFTp", [128, 2 * A], DT, kind="ExternalInput")
    d_inpp = nc.dram_tensor("inpp", [128, 2 * BL], DT, kind="ExternalInput")
    d_st3 = nc.dram_tensor("st3", [1, S], DT, kind="ExternalInput")
    d_wnsa = nc.dram_tensor("wnsa", [128, 5 * S], DT, kind="ExternalInput")
    d_wnsb = nc.dram_tensor("wnsb", [128, 5 * S], DT, kind="ExternalInput")

    d_ns = nc.dram_tensor("ns_out", [BL, S], DT, kind="ExternalOutput")
    d_fql = nc.dram_tensor("fql_out", [BL, A + 1], DT, kind="ExternalOutput")
    d_actp = nc.dram_tensor("actp_out", [128, 2 * BL], DT, kind="ExternalOutput")

    with tile.TileContext(nc) as tc:
        with (
            tc.tile_pool(name="const", bufs=1) as cpool,
            tc.tile_pool(name="work", bufs=2) as wpool,
            tc.tile_pool(name="fat", bufs=3) as fatpool,
            tc.tile_pool(name="psum", bufs=6, space="PSUM") as ppool,
            tc.tile_pool(name="psum_ns", bufs=1, space="PSUM") as ppool_ns,
        ):
            # ---- input DMAs: one per panel, in order of need -----------
            pk = cpool.tile([128, 2 * 4 * I], DT, tag="pk")
            nc.sync.dma_start(out=pk[:], in_=d_pk[:])
            selw = cpool.tile([128, 4 * A], DT, tag="selw")
            nc.sync.dma_start(out=selw[:], in_=d_selw[:])
            prevp = cpool.tile([128, 4 * BL], DT, tag="prevp")
            nc.sync.dma_start(out=prevp[:], in_=d_prevp[:])
            # the rest goes on the ACT HWDGE ring, in parallel
            sawma = cpool.tile([128, 5], DT, tag="sawma")
            nc.scalar.dma_start(out=sawma[:], in_=d_sawma[:])
            seld = cpool.tile([128, 4 * A], DT, tag="seld")
            nc.scalar.dma_start(out=seld[:], in_=d_seld[:])
            stFTp = cpool.tile([128, 2 * A], DT, tag="stFTp")
            nc.scalar.dma_start(out=stFTp[:], in_=d_stFTp[:])
            inpp = cpool.tile([128, 2 * BL], DT, tag="inpp")
            nc.scalar.dma_start(out=inpp[:], in_=d_inpp[:])
            st3 = cpool.tile([1, S], DT, tag="st3")
            nc.scalar.dma_start(out=st3[:], in_=d_st3[:])
            wnsa = cpool.tile([128, 5 * S], DT, tag="wnsa")
            nc.scalar.dma_start(out=wnsa[:], in_=d_wnsa[:])
            wnsb = cpool.tile([128, 5 * S], DT, tag="wnsb")
            nc.scalar.dma_start(out=wnsb[:], in_=d_wnsb[:])

            def prevT(k):
                return prevp[:, k * BL:(k + 1) * BL]

            def selwk(k):  # [128, 256] K-tile of select_w
                return selw[:, k * A:(k + 1) * A]

            def seldk(k):
                return seld[:, k * A:(k + 1) * A]

            def wns(k):
                if k < 5:
                    return wnsa[:, k * S:(k + 1) * S]
                return wnsb[:, (k - 5) * S:(k - 5 + 1) * S]

            ident = cpool.tile([BL, BL], DT, tag="ident")
            make_identity(nc, ident[:])
            ones_row = cpool.tile([1, BL], DT, tag="ones_row")
            nc.vector.memset(ones_row[:], 1.0)
            ma_col = sawma[:, 4:5]
            bias_hpi = cpool.tile([128, 1], DT, tag="bias_hpi")
            nc.vector.memset(bias_hpi[:], PI / 2)

            # ---- param prep, all on the DVE ------------------------------
            # sq2 = pk^2 ; per (half, param) slices are [128, 64]
            sq2 = wpool.tile([128, 2 * 4 * I], DT, tag="sq2")
            nc.vector.tensor_tensor(sq2[:], pk[:], pk[:], ALU.mult)

            def sqs(half, j):
                o = half * 4 * I + j * I
                return sq2[:, o:o + I], sq2[:, o + I - 1:o + I]  # slice, last col

            # w_plus = diff(wp2) (bf16), "wmb" = +diff(wm2) = -w_minus (bf16),
            # c = exclusive-cumsum(bp2), c' = exclusive-cumsum(bm2) (fp32)
            wpb, wmb, cK, cKm, W63p, W63m = [], [], [], [], [], []

            def prep_half(half):
                wp2, wp2last = sqs(half, 0)
                bp2, _ = sqs(half, 1)
                wm2, wm2last = sqs(half, 2)
                bm2, _ = sqs(half, 3)
                W63p.append(wp2last)
                W63m.append(wm2last)  # positive; sign handled in finish_half

                t = wpool.tile([128, I], BF, tag=f"wpb{half}")
                nc.vector.tensor_copy(t[:, 0:1], wp2[:, 0:1])
                nc.vector.tensor_tensor(t[:, 1:I], wp2[:, 1:I], wp2[:, 0:I - 1],
                                        ALU.subtract)
                wpb.append(t)
                t = wpool.tile([128, I], BF, tag=f"wmb{half}")
                nc.vector.tensor_copy(t[:, 0:1], wm2[:, 0:1])
                nc.vector.tensor_tensor(t[:, 1:I], wm2[:, 1:I], wm2[:, 0:I - 1],
                                        ALU.subtract)
                wmb.append(t)
                for src2, lst in ((bp2, cK), (bm2, cKm)):
                    s = wpool.tile([128, I], DT, tag="scan")
                    nc.vector.tensor_tensor_scan(s[:], src2, src2, 0.0,
                                                 ALU.add, ALU.bypass)
                    cc = wpool.tile([128, I], DT, tag=f"c{len(lst)}_{half}")
                    nc.vector.tensor_tensor(cc[:], s[:], src2, ALU.subtract)
                    lst.append(cc)
                # x2-expanded bf16 copies for the paired-innermost fat passes
                for nm2, (src_t, lst) in (("wp", (wpb[half], wpb2)),
                                          ("wm", (wmb[half], wmb2)),
                                          ("cp", (cK[half], cK2)),
                                          ("cm", (cKm[half], cKm2))):
                    e = wpool.tile([128, 2 * I], BF, tag=f"x2_{nm2}_{half}")
                    nc.vector.tensor_copy(
                        e[:].rearrange("p (i t) -> p i t", t=2),
                        src_t[:].rearrange("p (i u) -> p i u", u=1)
                        .broadcast_to([128, I, 2]))
                    lst.append(e)

            wpb2, wmb2, cK2, cKm2 = [], [], [], []
            prep_half(0)

            # ---- freq / delta:  fdT = sel^T-as-lhsT @ prevT --------------
            fd_ps = []
            for m in range(4):
                ps = ppool.tile([128, BL], DT, tag="ps")
                for k in range(4):
                    lhs = (selwk(k) if m < 2 else seldk(k))
                    mm = m % 2
                    nc.tensor.matmul(ps[:], lhs[:, mm * 128:(mm + 1) * 128],
                                     prevT(k), start=(k == 0), stop=(k == 3))
                fd_ps.append(ps)

            # r+ = relu(f), r- = relu(-f)  (fp32, straight from PSUM)
            r_p, r_m, rb_p, rb_m = [], [], [], []
            for half in range(2):
                rp = wpool.tile([128, BL], DT, tag=f"r_p{half}")
                nc.vector.tensor_scalar(rp[:], fd_ps[half][:], 0.0, None, ALU.max)
                rm = wpool.tile([128, BL], DT, tag=f"r_m{half}")
                nc.vector.tensor_scalar(rm[:], fd_ps[half][:], -1.0, 0.0,
                                        ALU.mult, ALU.max)
                r_p.append(rp)
                r_m.append(rm)
                rbp = wpool.tile([128, BL], BF, tag=f"rb_p{half}")
                nc.vector.tensor_copy(rbp[:], rp[:])
                rbm = wpool.tile([128, BL], BF, tag=f"rb_m{half}")
                nc.vector.tensor_copy(rbm[:], rm[:])
                rb_p.append(rbp)
                rb_m.append(rbm)
            prep_half(1)

            # ---- basis fat-pass helper ----------------------------------
            # Fat layout [p][b_hi=32][i=64][b_lo=2]: every operand gets a
            # dense step-1 innermost pair, which keeps the DVE in 2x mode
            # (a plain [b][i] layout leaves one operand with a stride-0
            # innermost -> 1x).  b = 2*b_hi + b_lo stays in natural order.
            def fat_unit(rb, r, w2, c2, w63):
                r_bc = (rb[:].rearrange("p (bh bl) -> p bh bl", bl=2)
                        .rearrange("p bh (u bl) -> p bh u bl", u=1)
                        .broadcast_to([128, BL // 2, I, 2]))
                c_bc = (c2[:].rearrange("p (i bl) -> p i bl", bl=2)
                        .rearrange("p (u i) bl -> p u i bl", u=1)
                        .broadcast_to([128, BL // 2, I, 2]))
                w_bc = (w2[:].rearrange("p (i bl) -> p i bl", bl=2)
                        .rearrange("p (u i) bl -> p u i bl", u=1)
                        .broadcast_to([128, BL // 2, I, 2]))
                M = fatpool.tile([128, BL * I], BF, tag="M")
                M4 = M[:].rearrange("p (bh i bl) -> p bh i bl", i=I, bl=2)
                nc.vector.tensor_tensor(M4, r_bc, c_bc, ALU.min)
                Y = fatpool.tile([128, BL * I], BF, tag="Y")
                Y4 = Y[:].rearrange("p (bh i bl) -> p bh i bl", i=I, bl=2)
                nc.vector.tensor_tensor(Y4, M4, w_bc, ALU.mult)
                cur, cur_i = Y4, I
                while cur_i > 2:
                    ni = cur_i // 2
                    F = fatpool.tile([128, BL * ni], BF, tag=f"F{ni}")
                    F4 = F[:].rearrange("p (bh i bl) -> p bh i bl", i=ni, bl=2)
                    nc.vector.tensor_tensor(F4, cur[:, :, 0:ni, :],
                                            cur[:, :, ni:cur_i, :], ALU.add)
                    cur, cur_i = F4, ni
                red = wpool.tile([128, BL], DT, tag="red")
                red4 = red[:].rearrange("p (bh i bl) -> p bh i bl", i=1, bl=2)
                nc.vector.tensor_tensor(red4, cur[:, :, 0:1, :],
                                        cur[:, :, 1:2, :], ALU.add)
                t = wpool.tile([128, BL], DT, tag="tbr")
                nc.vector.scalar_tensor_tensor(t[:], r[:], w63, red[:],
                                               ALU.mult, ALU.subtract)
                return t

            actp = wpool.tile([128, 2 * BL], DT, tag="actp")
            actT = []

            def finish_half(half, tp, tm):
                # anc = tp - tm  (tm carries +diff weights = -w_minus terms)
                anc = wpool.tile([128, BL], DT, tag="anc")
                nc.vector.tensor_tensor(anc[:], tp[:], tm[:], ALU.subtract)
                at = actp[:, half * BL:(half + 1) * BL]
                nc.vector.tensor_scalar(at, anc[:], nma_col[:], ma_col,
                                        ALU.max, ALU.min)
                actT.append(at)

            # FAT unit (+,0) goes first so the DVE gets busy asap
            t_p0 = fat_unit(rb_p[0], r_p[0], wpb2[0], cK2[0], W63p[0])
            nma_col = cpool.tile([128, 1], DT, tag="nma")
            nc.vector.tensor_scalar(nma_col[:], ma_col, -1.0, None, ALU.mult)

            # ---- sin / cos of delta (range-reduced; fills DVE/ACT gaps) --
            # y = x - 2pi*k via an int32 cast (round-to-nearest on HW,
            # trunc in CoreSim); a branch-free +-2pi correction makes the
            # result [-pi, pi] under either conversion mode.
            sinT, cosT = [], []
            for half in range(2):
                ki = wpool.tile([128, BL], mybir.dt.int32, tag="sc_ki")
                nc.vector.tensor_scalar(ki[:], fd_ps[2 + half][:],
                                        float(1 / (2 * PI)), 32.0, ALU.mult, ALU.add)
                xoff = wpool.tile([128, BL], DT, tag="sc_xoff")
                nc.vector.tensor_scalar(xoff[:], fd_ps[2 + half][:], float(64 * PI),
                                        None, ALU.add)
                y1 = wpool.tile([128, BL], DT, tag="sc_y1")
                nc.vector.scalar_tensor_tensor(y1[:], ki[:], float(-2 * PI), xoff[:],
                                               ALU.mult, ALU.add)
                w = wpool.tile([128, BL], DT, tag="sc_w")
                nc.vector.tensor_scalar(w[:], y1[:], PI, float(-2 * PI),
                                        ALU.is_gt, ALU.mult)
                y2 = wpool.tile([128, BL], DT, tag="sc_y2")
                nc.vector.tensor_tensor(y2[:], y1[:], w[:], ALU.add)
                y = wpool.tile([128, BL], DT, tag="sc_y")
                nc.vector.tensor_scalar(y[:], y2[:], -PI, PI, ALU.max, ALU.min)
                s = wpool.tile([128, BL], DT, tag=f"sinT{half}")
                nc.scalar.activation(s[:], y[:], ACTF.Sin)
                # cos(y) = sin(pi/2 - |y|),  argument stays in [-pi/2, pi/2]
                ay = wpool.tile([128, BL], DT, tag="sc_ay")
                nc.scalar.activation(ay[:], y[:], ACTF.Abs)
                c = wpool.tile([128, BL], DT, tag=f"cosT{half}")
                nc.scalar.activation(c[:], ay[:], ACTF.Sin, bias=bias_hpi[:],
                                     scale=-1.0)
                sinT.append(s)
                cosT.append(c)

            t_m0 = fat_unit(rb_m[0], r_m[0], wmb2[0], cKm2[0], W63m[0])
            finish_half(0, t_p0, t_m0)

            # ---- U = F @ cos, V = F @ sin ; dtT = sin*U - cos*V ----------
            dtT = []
            for m in range(2):
                psU = ppool.tile([128, BL], DT, tag="ps")
                psV = ppool.tile([128, BL], DT, tag="ps")
                for k in range(2):
                    lhs = stFTp[:, k * A + m * 128:k * A + (m + 1) * 128]
                    nc.tensor.matmul(psU[:], lhs, cosT[k][:], start=(k == 0), stop=(k == 1))
                    uv_last = nc.tensor.matmul(psV[:], lhs, sinT[k][:],
                                               start=(k == 0), stop=(k == 1))
                t1 = wpool.tile([128, BL], DT, tag="dt_t1")
                nc.vector.tensor_tensor(t1[:], sinT[m][:], psU[:], ALU.mult)
                t2 = wpool.tile([128, BL], DT, tag="dt_t2")
                nc.vector.tensor_tensor(t2[:], cosT[m][:], psV[:], ALU.mult)
                t = wpool.tile([128, BL], DT, tag=f"dtT{m}")
                nc.vector.tensor_tensor(t[:], t1[:], t2[:], ALU.subtract)
                dtT.append(t)

            # ---- new_state stacked matmul: everything that doesn't need
            # action accumulates into PSUM while the basis runs -------------
            ns_ps = ppool_ns.tile([BL, S], DT, tag="ns_ps")
            ns_first = nc.tensor.matmul(ns_ps[:], ones_row[:], st3[:],
                                        start=True, stop=False)
            tile.add_dep_helper(ns_first.ins, uv_last.ins, sync=False,
                                reason="keep PE free for U/V before the wns stack")
            for k in range(4):
                nc.tensor.matmul(ns_ps[:], prevT(k), wns(k), start=False, stop=False)
            for k in range(2):
                nc.tensor.matmul(ns_ps[:], inpp[:, k * BL:(k + 1) * BL], wns(6 + k),
                                 start=False, stop=False)
            for k in range(2):
                nc.tensor.matmul(ns_ps[:], dtT[k][:], wns(4 + k),
                                 start=False, stop=False)
            # action half 0 as soon as it exists (wns block 8 = st4 rows 0:128)
            nc.tensor.matmul(ns_ps[:], actT[0], wns(8), start=False, stop=False)

            # remaining fat units
            t_p1 = fat_unit(rb_p[1], r_p[1], wpb2[1], cK2[1], W63p[1])
            t_m1 = fat_unit(rb_m[1], r_m[1], wmb2[1], cKm2[1], W63m[1])
            finish_half(1, t_p1, t_m1)
            nc.sync.dma_start(out=d_actp[:], in_=actp[:])
            nc.tensor.matmul(ns_ps[:], actT[1], wns(9), start=False, stop=True)

            ns_nat = wpool.tile([BL, S], DT, tag="ns_nat")
            nc.vector.tensor_copy(ns_nat[:], ns_ps[:])
            nc.sync.dma_start(out=d_ns[:], in_=ns_nat[:])

            # ---- transpose new_state back to [s, b] for the out matmuls --
            nsT, ns2T = [], []
            for k in range(4):
                ps = ppool.tile([128, BL], DT, tag="ps")
                nc.tensor.transpose(ps[:], ns_nat[:, k * 128:(k + 1) * 128], ident[:])
                t = wpool.tile([128, BL], DT, tag=f"nsT{k}")
                nc.vector.tensor_copy(t[:], ps[:])
                nsT.append(t)
                t2 = wpool.tile([128, BL], DT, tag=f"ns2T{k}")
                nc.vector.tensor_tensor(t2[:], t[:], t[:], ALU.mult)
                ns2T.append(t2)

            # ---- frequency = ns @ select_w ; loss0 = ns^2 @ saw ----------
            fq_ps = ppool.tile([BL, A], DT, tag="ps")
            for k in range(4):
                nc.tensor.matmul(fq_ps[:], nsT[k][:], selwk(k),
                                 start=(k == 0), stop=(k == 3))
            fql = wpool.tile([BL, A + 1], DT, tag="fql")
            nc.vector.tensor_copy(fql[:, 0:A], fq_ps[:])

            ls_ps = ppool.tile([BL, 1], DT, tag="ps")
            for k in range(4):
                nc.tensor.matmul(ls_ps[:], ns2T[k][:], sawma[:, k:k + 1],
                                 start=(k == 0), stop=(k == 3))
            nc.vector.tensor_copy(fql[:, A:A + 1], ls_ps[:])
            nc.sync.dma_start(out=d_fql[:], in_=fql[:])

    nc.compile()
    return nc


_NC_CACHE = None


def _get_nc():
    global _NC_CACHE
    if _NC_CACHE is None:
        _NC_CACHE = build_nc()
    return _NC_CACHE


def _pack_rows(a, width):
    """[R, C] with R = 128*n  ->  [128, n*C] panel (blocks along free dim)."""
    r, c = a.shape
    n = r // 128
    return np.ascontiguousarray(
        a.reshape(n, 128, c).transpose(1, 0, 2).reshape(128, n * c))


def prepare_in_maps(inputs):
    x = {k: np.asarray(v) for k, v in inputs.items()}
    f32 = lambda a: np.ascontiguousarray(a, dtype=np.float32)

    wns = np.concatenate([x["state_transfer1"], x["state_transfer2"],
                          x["state_transfer3_Pm"], x["state_transfer4"]], axis=0)
    sawma = np.zeros((128, 5), np.float32)
    sawma[:, 0:4] = x["select_add_w"].reshape(4, 128).T
    sawma[:, 4] = np.float32(x["max_action"])

    shared = {
        "pk": f32(_pack_rows(np.concatenate(
            [x["w_plus_temp0"], x["b_plus_temp0"],
             x["w_minus_temp0"], x["b_minus_temp0"]], axis=1), 4 * I)),
        "selwp": f32(_pack_rows(x["select_w"], A)),
        "seldp": f32(_pack_rows(x["select_delta"], A)),
        "sawma": sawma,
        "stFTp": f32(_pack_rows(x["state_transferF"].T, A)),
        "st3": f32(x["state_transfer3"][None, :]),
        "wnsa": f32(_pack_rows(wns[:640], S)),
        "wnsb": f32(_pack_rows(wns[640:], S)),
    }
    in_maps = []
    for k in range(NCORES):
        sl = slice(k * BL, (k + 1) * BL)
        m = dict(shared)
        m["prevp"] = f32(_pack_rows(x["prev_output"][sl].T, BL))
        m["inpp"] = f32(_pack_rows(x["inputs"][sl].T, BL))
        in_maps.append(m)
    return in_maps


def postprocess(res):
    loss0 = np.concatenate([res[k]["fql_out"][:, A:A + 1] for k in range(NCORES)], axis=0)
    frequency = np.concatenate([res[k]["fql_out"][:, 0:A] for k in range(NCORES)], axis=0)
    action = np.concatenate(
        [np.concatenate([res[k]["actp_out"][:, 0:BL],
                         res[k]["actp_out"][:, BL:2 * BL]], axis=0).T
         for k in range(NCORES)], axis=0)
    new_state = np.concatenate([res[k]["ns_out"] for k in range(NCORES)], axis=0)
    return (loss0, frequency, action, new_state)


def kernel(**inputs):
    in_maps = prepare_in_maps(inputs)
    res = run_bass_kernel_spmd(_get_nc(), in_maps, list(range(NCORES))).results
    return postprocess(res)


# revision 21
# speedup vs baseline: 1.2424x; 1.0066x over previous
"""Trainium2 Bass kernel for nn_CustomRNNCell (Kuramoto-style RNN cell).

Strategy: pure data parallelism over the batch dim (B=512 -> 64 rows/core,
8 cores), parameters replicated.  All activations live on-chip in a
"transposed" layout [feature, batch] so every weight matrix is consumed by
the PE untransposed; the host does the input transposes / output
un-transposes / tensor packing (pure data movement).

Key algebraic restructurings (validated against the reference):
  * w_recover / b_recover are difference / strict-cumsum operators, so the
    basis-parameter prep is a square + shifted subtract + prefix scan on the
    DVE (no matmuls), and W63 = sum_i w_i is just the last column of wp^2.
  * delta_term = sin(d)*(F @ cos(d)) - cos(d)*(F @ sin(d))   (angle-difference
    expansion; kills the [B,A,A] sin grid)
  * piecewise-linear basis: with c = cumsum(b_t0^2) (knots, increasing),
      sum_i w_i*relu(f - c_i) = W63*relu(f) - sum_i w_i*min(c_i, relu(f))
    (the sum-w*c constants cancel between the two relu branches), computed
    with broadcast access patterns on the DVE; the fat min/mult/fold passes
    only ever see values <= c_max ~ 0.6 so they run in bf16, while the
    dominant W63*relu(f) term stays fp32.
  * clip(x,-m,m) = min(max(x,-m),m) as one tensor_scalar op.
  * new_state via one PSUM-accumulated matmul over the stacked
    [1; prev; inputs; delta_term; action] x [st3; st1; st3_Pm; st2; st4];
    everything not needing `action` accumulates while the basis runs.
  * params are host-packed into a handful of [128, N] panels -> one DMA each
    (the HWDGE ring serializes DMAs at ~0.6us apiece), split across the two
    HWDGE rings (sync + scalar).
"""

import sys

for _p in ("/opt/trn_rl_repo",):
    if _p not in sys.path:
        sys.path.insert(0, _p)

import numpy as np

import concourse.bacc as bacc
import concourse.mybir as mybir
import concourse.tile as tile
from concourse.bass_utils import run_bass_kernel_spmd
from concourse.masks import make_identity

B, A, I = 512, 256, 64
S, P = 512, 256
NCORES = 8
BL = B // NCORES  # 64 batch rows per core

DT = mybir.dt.float32
BF = mybir.dt.bfloat16
AX = mybir.AxisListType
ALU = mybir.AluOpType
ACTF = mybir.ActivationFunctionType

PI = float(np.pi)


def build_nc():
    nc = bacc.Bacc()

    # ---- DRAM I/O (host-packed panels) -------------------------------
    # pk: [128, (half, param, i)] = natural-layout wp_t0|bp_t0|wm_t0|bm_t0
    d_pk = nc.dram_tensor("pk", [128, 2 * 4 * I], DT, kind="ExternalInput")
    d_prevp = nc.dram_tensor("prevp", [128, 4 * BL], DT, kind="ExternalInput")
    d_selw = nc.dram_tensor("selwp", [128, 4 * A], DT, kind="ExternalInput")
    d_seld = nc.dram_tensor("seldp", [128, 4 * A], DT, kind="ExternalInput")
    d_sawma = nc.dram_tensor("sawma", [128, 5], DT, kind="ExternalInput")
    d_stFTp = nc.dram_tensor("stFTp", [128, 2 * A], DT, kind="ExternalInput")
    d_inpp = nc.dram_tensor("inpp", [128, 2 * BL], DT, kind="ExternalInput")
    d_st3 = nc.dram_tensor("st3", [1, S], DT, kind="ExternalInput")
    d_wnsa = nc.dram_tensor("wnsa", [128, 5 * S], DT, kind="ExternalInput")
    d_wnsb = nc.dram_tensor("wnsb", [128, 5 * S], DT, kind="ExternalInput")

    d_ns = nc.dram_tensor("ns_out", [BL, S], DT, kind="ExternalOutput")
    d_fql = nc.dram_tensor("fql_out", [BL, A + 1], DT, kind="ExternalOutput")
    d_actp = nc.dram_tensor("actp_out", [128, 2 * BL], DT, kind="ExternalOutput")

    with tile.TileContext(nc) as tc:
        with (
            tc.tile_pool(name="const", bufs=1) as cpool,
            tc.tile_pool(name="work", bufs=2) as wpool,
            tc.tile_pool(name="fat", bufs=3) as fatpool,
            tc.tile_pool(name="psum", bufs=6, space="PSUM") as ppool,
            tc.tile_pool(name="psum_ns", bufs=1, space="PSUM") as ppool_ns,
        ):
            # ---- input DMAs: one per panel, in order of need -----------
            pk = cpool.tile([128, 2 * 4 * I], DT, tag="pk")
            nc.sync.dma_start(out=pk[:], in_=d_pk[:])
            selw = cpool.tile([128, 4 * A], DT, tag="selw")
            nc.sync.dma_start(out=selw[:], in_=d_selw[:])
            prevp = cpool.tile([128, 4 * BL], DT, tag="prevp")
            nc.sync.dma_start(out=prevp[:], in_=d_prevp[:])
            # the rest goes on the ACT HWDGE ring, in parallel
            sawma = cpool.tile([128, 5], DT, tag="sawma")
            nc.scalar.dma_start(out=sawma[:], in_=d_sawma[:])
            seld = cpool.tile([128, 4 * A], DT, tag="seld")
            nc.scalar.dma_start(out=seld[:], in_=d_seld[:])
            stFTp = cpool.tile([128, 2 * A], DT, tag="stFTp")
            nc.scalar.dma_start(out=stFTp[:], in_=d_stFTp[:])
            inpp = cpool.tile([128, 2 * BL], DT, tag="inpp")
            nc.scalar.dma_start(out=inpp[:], in_=d_inpp[:])
            st3 = cpool.tile([1, S], DT, tag="st3")
            nc.scalar.dma_start(out=st3[:], in_=d_st3[:])
            wnsa = cpool.tile([128, 5 * S], DT, tag="wnsa")
            nc.scalar.dma_start(out=wnsa[:], in_=d_wnsa[:])
            wnsb = cpool.tile([128, 5 * S], DT, tag="wnsb")
            nc.scalar.dma_start(out=wnsb[:], in_=d_wnsb[:])

            def prevT(k):
                return prevp[:, k * BL:(k + 1) * BL]

            def selwk(k):  # [128, 256] K-tile of select_w
                return selw[:, k * A:(k + 1) * A]

            def seldk(k):
                return seld[:, k * A:(k + 1) * A]

            def wns(k):
                if k < 5:
                    return wnsa[:, k * S:(k + 1) * S]
                return wnsb[:, (k - 5) * S:(k - 5 + 1) * S]

            ident = cpool.tile([BL, BL], DT, tag="ident")
            make_identity(nc, ident[:])
            ones_row = cpool.tile([1, BL], DT, tag="ones_row")
            nc.vector.memset(ones_row[:], 1.0)
            ma_col = sawma[:, 4:5]
            bias_hpi = cpool.tile([128, 1], DT, tag="bias_hpi")
            nc.vector.memset(bias_hpi[:], PI / 2)

            # ---- param prep, all on the DVE ------------------------------
            # sq2 = pk^2 ; per (half, param) slices are [128, 64]
            sq2 = wpool.tile([128, 2 * 4 * I], DT, tag="sq2")
            nc.vector.tensor_tensor(sq2[:], pk[:], pk[:], ALU.mult)

            def sqs(half, j):
                o = half * 4 * I + j * I
                return sq2[:, o:o + I], sq2[:, o + I - 1:o + I]  # slice, last col

            # w_plus = diff(wp2) (bf16), "wmb" = +diff(wm2) = -w_minus (bf16),
            # c = exclusive-cumsum(bp2), c' = exclusive-cumsum(bm2) (fp32)
            wpb, wmb, cK, cKm, W63p, W63m = [], [], [], [], [], []

            def prep_half(half):
                wp2, wp2last = sqs(half, 0)
                bp2, _ = sqs(half, 1)
                wm2, wm2last = sqs(half, 2)
                bm2, _ = sqs(half, 3)
                W63p.append(wp2last)
                W63m.append(wm2last)  # positive; sign handled in finish_half

                t = wpool.tile([128, I], BF, tag=f"wpb{half}")
                nc.vector.tensor_copy(t[:, 0:1], wp2[:, 0:1])
                nc.vector.tensor_tensor(t[:, 1:I], wp2[:, 1:I], wp2[:, 0:I - 1],
                                        ALU.subtract)
                wpb.append(t)
                t = wpool.tile([128, I], BF, tag=f"wmb{half}")
                nc.vector.tensor_copy(t[:, 0:1], wm2[:, 0:1])
                nc.vector.tensor_tensor(t[:, 1:I], wm2[:, 1:I], wm2[:, 0:I - 1],
                                        ALU.subtract)
                wmb.append(t)
                for src2, lst in ((bp2, cK), (bm2, cKm)):
                    s = wpool.tile([128, I], DT, tag="scan")
                    nc.vector.tensor_tensor_scan(s[:], src2, src2, 0.0,
                                                 ALU.add, ALU.bypass)
                    cc = wpool.tile([128, I], DT, tag=f"c{len(lst)}_{half}")
                    nc.vector.tensor_tensor(cc[:], s[:], src2, ALU.subtract)
                    lst.append(cc)
                # x2-expanded bf16 copies for the paired-innermost fat passes
                for nm2, (src_t, lst) in (("wp", (wpb[half], wpb2)),
                                          ("wm", (wmb[half], wmb2)),
                                          ("cp", (cK[half], cK2)),
                                          ("cm", (cKm[half], cKm2))):
                    e = wpool.tile([128, 2 * I], BF, tag=f"x2_{nm2}_{half}")
                    nc.vector.tensor_copy(
                        e[:].rearrange("p (i t) -> p i t", t=2),
                        src_t[:].rearrange("p (i u) -> p i u", u=1)
                        .broadcast_to([128, I, 2]))
                    lst.append(e)

            wpb2, wmb2, cK2, cKm2 = [], [], [], []
            prep_half(0)

            # ---- freq / delta:  fdT = sel^T-as-lhsT @ prevT --------------
            fd_ps = []
            for m in range(4):
                ps = ppool.tile([128, BL], DT, tag="ps")
                for k in range(4):
                    lhs = (selwk(k) if m < 2 else seldk(k))
                    mm = m % 2
                    nc.tensor.matmul(ps[:], lhs[:, mm * 128:(mm + 1) * 128],
                                     prevT(k), start=(k == 0), stop=(k == 3))
                fd_ps.append(ps)

            # r+ = relu(f), r- = relu(-f)  (fp32, straight from PSUM)
            r_p, r_m, rb_p, rb_m = [], [], [], []
            for half in range(2):
                rp = wpool.tile([128, BL], DT, tag=f"r_p{half}")
                nc.vector.tensor_scalar(rp[:], fd_ps[half][:], 0.0, None, ALU.max)
                rm = wpool.tile([128, BL], DT, tag=f"r_m{half}")
                nc.vector.tensor_scalar(rm[:], fd_ps[half][:], -1.0, 0.0,
                                        ALU.mult, ALU.max)
                r_p.append(rp)
                r_m.append(rm)
                rbp = wpool.tile([128, BL], BF, tag=f"rb_p{half}")
                nc.vector.tensor_copy(rbp[:], rp[:])
                rbm = wpool.tile([128, BL], BF, tag=f"rb_m{half}")
                nc.vector.tensor_copy(rbm[:], rm[:])
                rb_p.append(rbp)
                rb_m.append(rbm)
            prep_half(1)

            # ---- basis fat-pass helper ----------------------------------
            # Fat layout [p][b_hi=32][i=64][b_lo=2]: every operand gets a
            # dense step-1 innermost pair, which keeps the DVE in 2x mode
            # (a plain [b][i] layout leaves one operand with a stride-0
            # innermost -> 1x).  b = 2*b_hi + b_lo stays in natural order.
            def fat_unit(rb, r, w2, c2, w63):
                r_bc = (rb[:].rearrange("p (bh bl) -> p bh bl", bl=2)
                        .rearrange("p bh (u bl) -> p bh u bl", u=1)
                        .broadcast_to([128, BL // 2, I, 2]))
                c_bc = (c2[:].rearrange("p (i bl) -> p i bl", bl=2)
                        .rearrange("p (u i) bl -> p u i bl", u=1)
                        .broadcast_to([128, BL // 2, I, 2]))
                w_bc = (w2[:].rearrange("p (i bl) -> p i bl", bl=2)
                        .rearrange("p (u i) bl -> p u i bl", u=1)
                        .broadcast_to([128, BL // 2, I, 2]))
                M = fatpool.tile([128, BL * I], BF, tag="M")
                M4 = M[:].rearrange("p (bh i bl) -> p bh i bl", i=I, bl=2)
                min_inst = nc.vector.tensor_tensor(M4, r_bc, c_bc, ALU.min)
                Y = fatpool.tile([128, BL * I], BF, tag="Y")
                Y4 = Y[:].rearrange("p (bh i bl) -> p bh i bl", i=I, bl=2)
                nc.vector.tensor_tensor(Y4, M4, w_bc, ALU.mult)
                cur, cur_i = Y4, I
                while cur_i > 2:
                    ni = cur_i // 2
                    F = fatpool.tile([128, BL * ni], BF, tag=f"F{ni}")
                    F4 = F[:].rearrange("p (bh i bl) -> p bh i bl", i=ni, bl=2)
                    nc.vector.tensor_tensor(F4, cur[:, :, 0:ni, :],
                                            cur[:, :, ni:cur_i, :], ALU.add)
                    cur, cur_i = F4, ni
                red = wpool.tile([128, BL], DT, tag="red")
                red4 = red[:].rearrange("p (bh i bl) -> p bh i bl", i=1, bl=2)
                red_inst = nc.vector.tensor_tensor(red4, cur[:, :, 0:1, :],
                                                   cur[:, :, 1:2, :], ALU.add)
                t = wpool.tile([128, BL], DT, tag="tbr")
                nc.vector.scalar_tensor_tensor(t[:], r[:], w63, red[:],
                                               ALU.mult, ALU.subtract)
                return t, min_inst, red_inst

            actp = wpool.tile([128, 2 * BL], DT, tag="actp")
            actT = []

            def pe_warm(gate_inst):
                # tiny matmul gated on basis progress so the PE's HAM clock
                # never re-throttles before the tail matmuls
                psd = ppool.tile([1, 16], DT, tag="ps")
                mm = nc.tensor.matmul(psd[:], ones_row[:, 0:1], st3[:, 0:16],
                                      start=True, stop=True)
                tile.add_dep_helper(mm.ins, gate_inst.ins, sync=True,
                                    reason="HAM warm-keeper")

            def finish_half(half, tp, tm):
                # anc = tp - tm  (tm carries +diff weights = -w_minus terms)
                anc = wpool.tile([128, BL], DT, tag="anc")
                nc.vector.tensor_tensor(anc[:], tp[:], tm[:], ALU.subtract)
                at = actp[:, half * BL:(half + 1) * BL]
                clip_inst = nc.vector.tensor_scalar(at, anc[:], nma_col[:], ma_col,
                                                    ALU.max, ALU.min)
                actT.append(at)
                return clip_inst

            # FAT unit (+,0) goes first so the DVE gets busy asap
            t_p0, minp0_i, redp0_i = fat_unit(rb_p[0], r_p[0], wpb2[0], cK2[0], W63p[0])
            nma_col = cpool.tile([128, 1], DT, tag="nma")
            nc.vector.tensor_scalar(nma_col[:], ma_col, -1.0, None, ALU.mult)

            # ---- sin / cos of delta (range-reduced; fills DVE/ACT gaps) --
            # y = x - 2pi*k via an int32 cast (round-to-nearest on HW,
            # trunc in CoreSim); a branch-free +-2pi correction makes the
            # result [-pi, pi] under either conversion mode.
            sinT, cosT = [], []
            for half in range(2):
                ki = wpool.tile([128, BL], mybir.dt.int32, tag="sc_ki")
                nc.vector.tensor_scalar(ki[:], fd_ps[2 + half][:],
                                        float(1 / (2 * PI)), 32.0, ALU.mult, ALU.add)
                xoff = wpool.tile([128, BL], DT, tag="sc_xoff")
                nc.vector.tensor_scalar(xoff[:], fd_ps[2 + half][:], float(64 * PI),
                                        None, ALU.add)
                y1 = wpool.tile([128, BL], DT, tag="sc_y1")
                nc.vector.scalar_tensor_tensor(y1[:], ki[:], float(-2 * PI), xoff[:],
                                               ALU.mult, ALU.add)
                w = wpool.tile([128, BL], DT, tag="sc_w")
                nc.vector.tensor_scalar(w[:], y1[:], PI, float(-2 * PI),
                                        ALU.is_gt, ALU.mult)
                y2 = wpool.tile([128, BL], DT, tag="sc_y2")
                nc.vector.tensor_tensor(y2[:], y1[:], w[:], ALU.add)
                y = wpool.tile([128, BL], DT, tag="sc_y")
                nc.vector.tensor_scalar(y[:], y2[:], -PI, PI, ALU.max, ALU.min)
                s = wpool.tile([128, BL], DT, tag=f"sinT{half}")
                nc.scalar.activation(s[:], y[:], ACTF.Sin)
                # cos(y) = sin(pi/2 - |y|),  argument stays in [-pi/2, pi/2]
                ay = wpool.tile([128, BL], DT, tag="sc_ay")
                nc.scalar.activation(ay[:], y[:], ACTF.Abs)
                c = wpool.tile([128, BL], DT, tag=f"cosT{half}")
                nc.scalar.activation(c[:], ay[:], ACTF.Sin, bias=bias_hpi[:],
                                     scale=-1.0)
                sinT.append(s)
                cosT.append(c)

            t_m0, minm0_i, redm0_i = fat_unit(rb_m[0], r_m[0], wmb2[0], cKm2[0], W63m[0])
            clip0_i = finish_half(0, t_p0, t_m0)
            pe_warm(redp0_i)
            pe_warm(redm0_i)

            # ---- U = F @ cos, V = F @ sin ; dtT = sin*U - cos*V ----------
            dtT = []
            for m in range(2):
                psU = ppool.tile([128, BL], DT, tag="ps")
                psV = ppool.tile([128, BL], DT, tag="ps")
                for k in range(2):
                    lhs = stFTp[:, k * A + m * 128:k * A + (m + 1) * 128]
                    nc.tensor.matmul(psU[:], lhs, cosT[k][:], start=(k == 0), stop=(k == 1))
                    uv_last = nc.tensor.matmul(psV[:], lhs, sinT[k][:],
                                               start=(k == 0), stop=(k == 1))
                t1 = wpool.tile([128, BL], DT, tag="dt_t1")
                nc.vector.tensor_tensor(t1[:], sinT[m][:], psU[:], ALU.mult)
                t2 = wpool.tile([128, BL], DT, tag="dt_t2")
                nc.vector.tensor_tensor(t2[:], cosT[m][:], psV[:], ALU.mult)
                t = wpool.tile([128, BL], DT, tag=f"dtT{m}")
                nc.vector.tensor_tensor(t[:], t1[:], t2[:], ALU.subtract)
                dtT.append(t)

            # ---- new_state stacked matmul: everything that doesn't need
            # action accumulates into PSUM while the basis runs -------------
            ns_ps = ppool_ns.tile([BL, S], DT, tag="ns_ps")
            ns_first = nc.tensor.matmul(ns_ps[:], ones_row[:], st3[:],
                                        start=True, stop=False)
            tile.add_dep_helper(ns_first.ins, uv_last.ins, sync=False,
                                reason="keep PE free for U/V before the wns stack")
            for k in range(4):
                nc.tensor.matmul(ns_ps[:], prevT(k), wns(k), start=False, stop=False)
            for k in range(2):
                nc.tensor.matmul(ns_ps[:], inpp[:, k * BL:(k + 1) * BL], wns(6 + k),
                                 start=False, stop=False)
            for k in range(2):
                nc.tensor.matmul(ns_ps[:], dtT[k][:], wns(4 + k),
                                 start=False, stop=False)
            # action half 0 as soon as it exists (wns block 8 = st4 rows 0:128)
            nc.tensor.matmul(ns_ps[:], actT[0], wns(8), start=False, stop=False)

            # remaining fat units
            t_p1, minp1_i, redp1_i = fat_unit(rb_p[1], r_p[1], wpb2[1], cK2[1], W63p[1])
            tile.add_dep_helper(minp1_i.ins, clip0_i.ins, sync=False,
                                reason="finish half-0 before unit 3 so action-half-0 matmul runs early")
            t_m1, minm1_i, redm1_i = fat_unit(rb_m[1], r_m[1], wmb2[1], cKm2[1], W63m[1])
            finish_half(1, t_p1, t_m1)
            pe_warm(redp1_i)
            nc.sync.dma_start(out=d_actp[:], in_=actp[:])
            nc.tensor.matmul(ns_ps[:], actT[1], wns(9), start=False, stop=True)

            ns_nat = wpool.tile([BL, S], DT, tag="ns_nat")
            nc.vector.tensor_copy(ns_nat[:], ns_ps[:])
            nc.sync.dma_start(out=d_ns[:], in_=ns_nat[:])

            # ---- transpose new_state back to [s, b] for the out matmuls --
            nsT, ns2T = [], []
            for k in range(4):
                ps = ppool.tile([128, BL], DT, tag="ps")
                nc.tensor.transpose(ps[:], ns_nat[:, k * 128:(k + 1) * 128], ident[:])
                t = wpool.tile([128, BL], DT, tag=f"nsT{k}")
                nc.vector.tensor_copy(t[:], ps[:])
                nsT.append(t)
                t2 = wpool.tile([128, BL], DT, tag=f"ns2T{k}")
                nc.vector.tensor_tensor(t2[:], t[:], t[:], ALU.mult)
                ns2T.append(t2)

            # ---- frequency = ns @ select_w ; loss0 = ns^2 @ saw ----------
            fq_ps = ppool.tile([BL, A], DT, tag="ps")
            for k in range(4):
                nc.tensor.matmul(fq_ps[:], nsT[k][:], selwk(k),
                                 start=(k == 0), stop=(k == 3))
            fql = wpool.tile([BL, A + 1], DT, tag="fql")
            nc.vector.tensor_copy(fql[:, 0:A], fq_ps[:])

            ls_ps = ppool.tile([BL, 1], DT, tag="ps")
            for k in range(4):
                nc.tensor.matmul(ls_ps[:], ns2T[k][:], sawma[:, k:k + 1],
                                 start=(k == 0), stop=(k == 3))
            nc.vector.tensor_copy(fql[:, A:A + 1], ls_ps[:])
            nc.sync.dma_start(out=d_fql[:], in_=fql[:])

    nc.compile()
    return nc


_NC_CACHE = None


def _get_nc():
    global _NC_CACHE
    if _NC_CACHE is None:
        _NC_CACHE = build_nc()
    return _NC_CACHE


def _pack_rows(a, width):
    """[R, C] with R = 128*n  ->  [128, n*C] panel (blocks along free dim)."""
    r, c = a.shape
    n = r // 128
    return np.ascontiguousarray(
        a.reshape(n, 128, c).transpose(1, 0, 2).reshape(128, n * c))


def prepare_in_maps(inputs):
    x = {k: np.asarray(v) for k, v in inputs.items()}
    f32 = lambda a: np.ascontiguousarray(a, dtype=np.float32)

    wns = np.concatenate([x["state_transfer1"], x["state_transfer2"],
                          x["state_transfer3_Pm"], x["state_transfer4"]], axis=0)
    sawma = np.zeros((128, 5), np.float32)
    sawma[:, 0:4] = x["select_add_w"].reshape(4, 128).T
    sawma[:, 4] = np.float32(x["max_action"])

    shared = {
        "pk": f32(_pack_rows(np.concatenate(
            [x["w_plus_temp0"], x["b_plus_temp0"],
             x["w_minus_temp0"], x["b_minus_temp0"]], axis=1), 4 * I)),
        "selwp": f32(_pack_rows(x["select_w"], A)),
        "seldp": f32(_pack_rows(x["select_delta"], A)),
        "sawma": sawma,
        "stFTp": f32(_pack_rows(x["state_transferF"].T, A)),
        "st3": f32(x["state_transfer3"][None, :]),
        "wnsa": f32(_pack_rows(wns[:640], S)),
        "wnsb": f32(_pack_rows(wns[640:], S)),
    }
    in_maps = []
    for k in range(NCORES):
        sl = slice(k * BL, (k + 1) * BL)
        m = dict(shared)
        m["prevp"] = f32(_pack_rows(x["prev_output"][sl].T, BL))
        m["inpp"] = f32(_pack_rows(x["inputs"][sl].T, BL))
        in_maps.append(m)
    return in_maps


def postprocess(res):
    loss0 = np.concatenate([res[k]["fql_out"][:, A:A + 1] for k in range(NCORES)], axis=0)
    frequency = np.concatenate([res[k]["fql_out"][:, 0:A] for k in range(NCORES)], axis=0)
    action = np.concatenate(
        [np.concatenate([res[k]["actp_out"][:, 0:BL],
                         res[k]["actp_out"][:, BL:2 * BL]], axis=0).T
         for k in range(NCORES)], axis=0)
    new_state = np.concatenate([res[k]["ns_out"] for k in range(NCORES)], axis=0)
    return (loss0, frequency, action, new_state)


def kernel(**inputs):
    in_maps = prepare_in_maps(inputs)
    res = run_bass_kernel_spmd(_get_nc(), in_maps, list(range(NCORES))).results
    return postprocess(res)


# revision 22
# speedup vs baseline: 1.2749x; 1.0261x over previous
"""Trainium2 Bass kernel for nn_CustomRNNCell (Kuramoto-style RNN cell).

Strategy: pure data parallelism over the batch dim (B=512 -> 64 rows/core,
8 cores), parameters replicated.  All activations live on-chip in a
"transposed" layout [feature, batch] so every weight matrix is consumed by
the PE untransposed; the host does the input transposes / output
un-transposes / tensor packing (pure data movement).

Key algebraic restructurings (validated against the reference):
  * w_recover / b_recover are difference / strict-cumsum operators, so the
    basis-parameter prep is a square + shifted subtract + prefix scan on the
    DVE (no matmuls), and W63 = sum_i w_i is just the last column of wp^2.
  * delta_term = sin(d)*(F @ cos(d)) - cos(d)*(F @ sin(d))   (angle-difference
    expansion; kills the [B,A,A] sin grid)
  * piecewise-linear basis: with c = cumsum(b_t0^2) (knots, increasing),
      sum_i w_i*relu(f - c_i) = W63*relu(f) - sum_i w_i*min(c_i, relu(f))
    (the sum-w*c constants cancel between the two relu branches), computed
    with broadcast access patterns on the DVE; the fat min/mult/fold passes
    only ever see values <= c_max ~ 0.6 so they run in bf16, while the
    dominant W63*relu(f) term stays fp32.
  * clip(x,-m,m) = min(max(x,-m),m) as one tensor_scalar op.
  * new_state via one PSUM-accumulated matmul over the stacked
    [1; prev; inputs; delta_term; action] x [st3; st1; st3_Pm; st2; st4];
    everything not needing `action` accumulates while the basis runs.
  * params are host-packed into a handful of [128, N] panels -> one DMA each
    (the HWDGE ring serializes DMAs at ~0.6us apiece), split across the two
    HWDGE rings (sync + scalar).
"""

import sys

for _p in ("/opt/trn_rl_repo",):
    if _p not in sys.path:
        sys.path.insert(0, _p)

import numpy as np

import concourse.bacc as bacc
import concourse.mybir as mybir
import concourse.tile as tile
from concourse.bass_utils import run_bass_kernel_spmd
from concourse.masks import make_identity

B, A, I = 512, 256, 64
S, P = 512, 256
NCORES = 8
BL = B // NCORES  # 64 batch rows per core

DT = mybir.dt.float32
BF = mybir.dt.bfloat16
AX = mybir.AxisListType
ALU = mybir.AluOpType
ACTF = mybir.ActivationFunctionType

PI = float(np.pi)


def build_nc():
    nc = bacc.Bacc()

    # ---- DRAM I/O (host-packed panels) -------------------------------
    # pk: [128, (half, param, i)] = natural-layout wp_t0|bp_t0|wm_t0|bm_t0
    d_pk = nc.dram_tensor("pk", [128, 2 * 4 * I], DT, kind="ExternalInput")
    d_prevp = nc.dram_tensor("prevp", [128, 4 * BL], DT, kind="ExternalInput")
    d_selw = nc.dram_tensor("selwp", [128, 4 * A], DT, kind="ExternalInput")
    d_seld = nc.dram_tensor("seldp", [128, 4 * A], DT, kind="ExternalInput")
    d_sawma = nc.dram_tensor("sawma", [128, 5], DT, kind="ExternalInput")
    d_stFTp = nc.dram_tensor("stFTp", [128, 2 * A], DT, kind="ExternalInput")
    d_inpp = nc.dram_tensor("inpp", [128, 2 * BL], DT, kind="ExternalInput")
    d_st3 = nc.dram_tensor("st3", [1, S], DT, kind="ExternalInput")
    d_wnsa = nc.dram_tensor("wnsa", [128, 5 * S], DT, kind="ExternalInput")
    d_wnsb = nc.dram_tensor("wnsb", [128, 5 * S], DT, kind="ExternalInput")

    d_ns = nc.dram_tensor("ns_out", [BL, S], DT, kind="ExternalOutput")
    d_fql = nc.dram_tensor("fql_out", [BL, A + 1], DT, kind="ExternalOutput")
    d_actp = nc.dram_tensor("actp_out", [128, 2 * BL], DT, kind="ExternalOutput")

    with tile.TileContext(nc) as tc:
        with (
            tc.tile_pool(name="const", bufs=1) as cpool,
            tc.tile_pool(name="work", bufs=2) as wpool,
            tc.tile_pool(name="fat", bufs=3) as fatpool,
            tc.tile_pool(name="psum", bufs=6, space="PSUM") as ppool,
            tc.tile_pool(name="psum_ns", bufs=1, space="PSUM") as ppool_ns,
        ):
            # ---- input DMAs: one per panel, in order of need -----------
            pk = cpool.tile([128, 2 * 4 * I], DT, tag="pk")
            nc.sync.dma_start(out=pk[:], in_=d_pk[:])
            selw = cpool.tile([128, 4 * A], DT, tag="selw")
            nc.sync.dma_start(out=selw[:], in_=d_selw[:])
            prevp = cpool.tile([128, 4 * BL], DT, tag="prevp")
            nc.sync.dma_start(out=prevp[:], in_=d_prevp[:])
            # the rest goes on the ACT HWDGE ring, in parallel
            sawma = cpool.tile([128, 5], DT, tag="sawma")
            nc.scalar.dma_start(out=sawma[:], in_=d_sawma[:])
            seld = cpool.tile([128, 4 * A], DT, tag="seld")
            nc.scalar.dma_start(out=seld[:], in_=d_seld[:])
            stFTp = cpool.tile([128, 2 * A], DT, tag="stFTp")
            nc.scalar.dma_start(out=stFTp[:], in_=d_stFTp[:])
            inpp = cpool.tile([128, 2 * BL], DT, tag="inpp")
            nc.scalar.dma_start(out=inpp[:], in_=d_inpp[:])
            st3 = cpool.tile([1, S], DT, tag="st3")
            nc.scalar.dma_start(out=st3[:], in_=d_st3[:])
            wnsa = cpool.tile([128, 5 * S], DT, tag="wnsa")
            nc.scalar.dma_start(out=wnsa[:], in_=d_wnsa[:])
            wnsb = cpool.tile([128, 5 * S], DT, tag="wnsb")
            nc.scalar.dma_start(out=wnsb[:], in_=d_wnsb[:])

            def prevT(k):
                return prevp[:, k * BL:(k + 1) * BL]

            def selwk(k):  # [128, 256] K-tile of select_w
                return selw[:, k * A:(k + 1) * A]

            def seldk(k):
                return seld[:, k * A:(k + 1) * A]

            def wns(k):
                if k < 5:
                    return wnsa[:, k * S:(k + 1) * S]
                return wnsb[:, (k - 5) * S:(k - 5 + 1) * S]

            ident = cpool.tile([BL, BL], DT, tag="ident")
            make_identity(nc, ident[:])
            ones_row = cpool.tile([1, BL], DT, tag="ones_row")
            nc.vector.memset(ones_row[:], 1.0)
            ma_col = sawma[:, 4:5]
            bias_hpi = cpool.tile([128, 1], DT, tag="bias_hpi")
            nc.vector.memset(bias_hpi[:], PI / 2)

            # ---- param prep, all on the DVE ------------------------------
            # sq2 = pk^2 ; per (half, param) slices are [128, 64]
            sq2 = wpool.tile([128, 2 * 4 * I], DT, tag="sq2")
            nc.vector.tensor_tensor(sq2[:], pk[:], pk[:], ALU.mult)

            def sqs(half, j):
                o = half * 4 * I + j * I
                return sq2[:, o:o + I], sq2[:, o + I - 1:o + I]  # slice, last col

            # w_plus = diff(wp2) (bf16), "wmb" = +diff(wm2) = -w_minus (bf16),
            # c = exclusive-cumsum(bp2), c' = exclusive-cumsum(bm2) (fp32)
            wpb, wmb, cK, cKm, W63p, W63m = [], [], [], [], [], []

            def prep_half(half):
                wp2, wp2last = sqs(half, 0)
                bp2, _ = sqs(half, 1)
                wm2, wm2last = sqs(half, 2)
                bm2, _ = sqs(half, 3)
                W63p.append(wp2last)
                W63m.append(wm2last)  # positive; sign handled in finish_half

                t = wpool.tile([128, I], BF, tag=f"wpb{half}")
                nc.vector.tensor_copy(t[:, 0:1], wp2[:, 0:1])
                nc.vector.tensor_tensor(t[:, 1:I], wp2[:, 1:I], wp2[:, 0:I - 1],
                                        ALU.subtract)
                wpb.append(t)
                t = wpool.tile([128, I], BF, tag=f"wmb{half}")
                nc.vector.tensor_copy(t[:, 0:1], wm2[:, 0:1])
                nc.vector.tensor_tensor(t[:, 1:I], wm2[:, 1:I], wm2[:, 0:I - 1],
                                        ALU.subtract)
                wmb.append(t)
                for src2, lst in ((bp2, cK), (bm2, cKm)):
                    s = wpool.tile([128, I], DT, tag="scan")
                    nc.vector.tensor_tensor_scan(s[:], src2, src2, 0.0,
                                                 ALU.add, ALU.bypass)
                    cc = wpool.tile([128, I], DT, tag=f"c{len(lst)}_{half}")
                    nc.vector.tensor_tensor(cc[:], s[:], src2, ALU.subtract)
                    lst.append(cc)
                # x2-expanded bf16 copies for the paired-innermost fat passes
                for nm2, (src_t, lst) in (("wp", (wpb[half], wpb2)),
                                          ("wm", (wmb[half], wmb2)),
                                          ("cp", (cK[half], cK2)),
                                          ("cm", (cKm[half], cKm2))):
                    e = wpool.tile([128, 2 * I], BF, tag=f"x2_{nm2}_{half}")
                    nc.vector.tensor_copy(
                        e[:].rearrange("p (i t) -> p i t", t=2),
                        src_t[:].rearrange("p (i u) -> p i u", u=1)
                        .broadcast_to([128, I, 2]))
                    lst.append(e)

            wpb2, wmb2, cK2, cKm2 = [], [], [], []
            prep_half(0)

            # ---- freq / delta:  fdT = sel^T-as-lhsT @ prevT --------------
            fd_ps = []
            for m in range(4):
                ps = ppool.tile([128, BL], DT, tag="ps")
                for k in range(4):
                    lhs = (selwk(k) if m < 2 else seldk(k))
                    mm = m % 2
                    nc.tensor.matmul(ps[:], lhs[:, mm * 128:(mm + 1) * 128],
                                     prevT(k), start=(k == 0), stop=(k == 3))
                fd_ps.append(ps)

            # r+ = relu(f), r- = relu(-f)  (fp32, straight from PSUM)
            r_p, r_m, rb_p, rb_m = [], [], [], []

            def r_half(half):
                rp = wpool.tile([128, BL], DT, tag=f"r_p{half}")
                nc.vector.tensor_scalar(rp[:], fd_ps[half][:], 0.0, None, ALU.max)
                rm = wpool.tile([128, BL], DT, tag=f"r_m{half}")
                nc.vector.tensor_scalar(rm[:], fd_ps[half][:], -1.0, 0.0,
                                        ALU.mult, ALU.max)
                r_p.append(rp)
                r_m.append(rm)
                rbp = wpool.tile([128, BL], BF, tag=f"rb_p{half}")
                nc.vector.tensor_copy(rbp[:], rp[:])
                rbm = wpool.tile([128, BL], BF, tag=f"rb_m{half}")
                nc.vector.tensor_copy(rbm[:], rm[:])
                rb_p.append(rbp)
                rb_m.append(rbm)

            r_half(0)

            # ---- basis fat-pass helper ----------------------------------
            # Fat layout [p][b_hi=32][i=64][b_lo=2]: every operand gets a
            # dense step-1 innermost pair, which keeps the DVE in 2x mode
            # (a plain [b][i] layout leaves one operand with a stride-0
            # innermost -> 1x).  b = 2*b_hi + b_lo stays in natural order.
            def fat_unit(rb, r, w2, c2, w63):
                r_bc = (rb[:].rearrange("p (bh bl) -> p bh bl", bl=2)
                        .rearrange("p bh (u bl) -> p bh u bl", u=1)
                        .broadcast_to([128, BL // 2, I, 2]))
                c_bc = (c2[:].rearrange("p (i bl) -> p i bl", bl=2)
                        .rearrange("p (u i) bl -> p u i bl", u=1)
                        .broadcast_to([128, BL // 2, I, 2]))
                w_bc = (w2[:].rearrange("p (i bl) -> p i bl", bl=2)
                        .rearrange("p (u i) bl -> p u i bl", u=1)
                        .broadcast_to([128, BL // 2, I, 2]))
                M = fatpool.tile([128, BL * I], BF, tag="M")
                M4 = M[:].rearrange("p (bh i bl) -> p bh i bl", i=I, bl=2)
                min_inst = nc.vector.tensor_tensor(M4, r_bc, c_bc, ALU.min)
                Y = fatpool.tile([128, BL * I], BF, tag="Y")
                Y4 = Y[:].rearrange("p (bh i bl) -> p bh i bl", i=I, bl=2)
                nc.vector.tensor_tensor(Y4, M4, w_bc, ALU.mult)
                cur, cur_i = Y4, I
                while cur_i > 2:
                    ni = cur_i // 2
                    F = fatpool.tile([128, BL * ni], BF, tag=f"F{ni}")
                    F4 = F[:].rearrange("p (bh i bl) -> p bh i bl", i=ni, bl=2)
                    nc.vector.tensor_tensor(F4, cur[:, :, 0:ni, :],
                                            cur[:, :, ni:cur_i, :], ALU.add)
                    cur, cur_i = F4, ni
                red = wpool.tile([128, BL], DT, tag="red")
                red4 = red[:].rearrange("p (bh i bl) -> p bh i bl", i=1, bl=2)
                red_inst = nc.vector.tensor_tensor(red4, cur[:, :, 0:1, :],
                                                   cur[:, :, 1:2, :], ALU.add)
                t = wpool.tile([128, BL], DT, tag="tbr")
                nc.vector.scalar_tensor_tensor(t[:], r[:], w63, red[:],
                                               ALU.mult, ALU.subtract)
                return t, min_inst, red_inst

            actp = wpool.tile([128, 2 * BL], DT, tag="actp")
            actT = []

            def pe_warm(gate_inst):
                # tiny matmul gated on basis progress so the PE's HAM clock
                # never re-throttles before the tail matmuls
                psd = ppool.tile([1, 16], DT, tag="ps")
                mm = nc.tensor.matmul(psd[:], ones_row[:, 0:1], st3[:, 0:16],
                                      start=True, stop=True)
                tile.add_dep_helper(mm.ins, gate_inst.ins, sync=True,
                                    reason="HAM warm-keeper")

            def finish_half(half, tp, tm):
                # anc = tp - tm  (tm carries +diff weights = -w_minus terms)
                anc = wpool.tile([128, BL], DT, tag="anc")
                nc.vector.tensor_tensor(anc[:], tp[:], tm[:], ALU.subtract)
                at = actp[:, half * BL:(half + 1) * BL]
                clip_inst = nc.vector.tensor_scalar(at, anc[:], nma_col[:], ma_col,
                                                    ALU.max, ALU.min)
                actT.append(at)
                return clip_inst

            # FAT unit (+,0) goes first so the DVE gets busy asap
            t_p0, minp0_i, redp0_i = fat_unit(rb_p[0], r_p[0], wpb2[0], cK2[0], W63p[0])
            r_half(1)
            prep_half(1)
            nma_col = cpool.tile([128, 1], DT, tag="nma")
            nc.vector.tensor_scalar(nma_col[:], ma_col, -1.0, None, ALU.mult)

            # ---- sin / cos of delta (range-reduced; fills DVE/ACT gaps) --
            # y = x - 2pi*k via an int32 cast (round-to-nearest on HW,
            # trunc in CoreSim); a branch-free +-2pi correction makes the
            # result [-pi, pi] under either conversion mode.
            sinT, cosT = [], []
            for half in range(2):
                ki = wpool.tile([128, BL], mybir.dt.int32, tag="sc_ki")
                nc.vector.tensor_scalar(ki[:], fd_ps[2 + half][:],
                                        float(1 / (2 * PI)), 32.0, ALU.mult, ALU.add)
                xoff = wpool.tile([128, BL], DT, tag="sc_xoff")
                nc.vector.tensor_scalar(xoff[:], fd_ps[2 + half][:], float(64 * PI),
                                        None, ALU.add)
                y1 = wpool.tile([128, BL], DT, tag="sc_y1")
                nc.vector.scalar_tensor_tensor(y1[:], ki[:], float(-2 * PI), xoff[:],
                                               ALU.mult, ALU.add)
                w = wpool.tile([128, BL], DT, tag="sc_w")
                nc.vector.tensor_scalar(w[:], y1[:], PI, float(-2 * PI),
                                        ALU.is_gt, ALU.mult)
                y2 = wpool.tile([128, BL], DT, tag="sc_y2")
                nc.vector.tensor_tensor(y2[:], y1[:], w[:], ALU.add)
                y = wpool.tile([128, BL], DT, tag="sc_y")
                nc.vector.tensor_scalar(y[:], y2[:], -PI, PI, ALU.max, ALU.min)
                s = wpool.tile([128, BL], DT, tag=f"sinT{half}")
                nc.scalar.activation(s[:], y[:], ACTF.Sin)
                # cos(y) = sin(pi/2 - |y|),  argument stays in [-pi/2, pi/2]
                ay = wpool.tile([128, BL], DT, tag="sc_ay")
                nc.scalar.activation(ay[:], y[:], ACTF.Abs)
                c = wpool.tile([128, BL], DT, tag=f"cosT{half}")
                nc.scalar.activation(c[:], ay[:], ACTF.Sin, bias=bias_hpi[:],
                                     scale=-1.0)
                sinT.append(s)
                cosT.append(c)

            t_m0, minm0_i, redm0_i = fat_unit(rb_m[0], r_m[0], wmb2[0], cKm2[0], W63m[0])
            clip0_i = finish_half(0, t_p0, t_m0)
            pe_warm(redp0_i)
            pe_warm(redm0_i)

            # ---- U = F @ cos, V = F @ sin ; dtT = sin*U - cos*V ----------
            dtT = []
            for m in range(2):
                psU = ppool.tile([128, BL], DT, tag="ps")
                psV = ppool.tile([128, BL], DT, tag="ps")
                for k in range(2):
                    lhs = stFTp[:, k * A + m * 128:k * A + (m + 1) * 128]
                    nc.tensor.matmul(psU[:], lhs, cosT[k][:], start=(k == 0), stop=(k == 1))
                    uv_last = nc.tensor.matmul(psV[:], lhs, sinT[k][:],
                                               start=(k == 0), stop=(k == 1))
                t1 = wpool.tile([128, BL], DT, tag="dt_t1")
                nc.vector.tensor_tensor(t1[:], sinT[m][:], psU[:], ALU.mult)
                t2 = wpool.tile([128, BL], DT, tag="dt_t2")
                nc.vector.tensor_tensor(t2[:], cosT[m][:], psV[:], ALU.mult)
                t = wpool.tile([128, BL], DT, tag=f"dtT{m}")
                nc.vector.tensor_tensor(t[:], t1[:], t2[:], ALU.subtract)
                dtT.append(t)

            # ---- new_state stacked matmul: everything that doesn't need
            # action accumulates into PSUM while the basis runs -------------
            ns_ps = ppool_ns.tile([BL, S], DT, tag="ns_ps")
            ns_first = nc.tensor.matmul(ns_ps[:], ones_row[:], st3[:],
                                        start=True, stop=False)
            tile.add_dep_helper(ns_first.ins, uv_last.ins, sync=False,
                                reason="keep PE free for U/V before the wns stack")
            for k in range(4):
                nc.tensor.matmul(ns_ps[:], prevT(k), wns(k), start=False, stop=False)
            for k in range(2):
                nc.tensor.matmul(ns_ps[:], inpp[:, k * BL:(k + 1) * BL], wns(6 + k),
                                 start=False, stop=False)
            for k in range(2):
                nc.tensor.matmul(ns_ps[:], dtT[k][:], wns(4 + k),
                                 start=False, stop=False)
            # action half 0 as soon as it exists (wns block 8 = st4 rows 0:128)
            nc.tensor.matmul(ns_ps[:], actT[0], wns(8), start=False, stop=False)

            # remaining fat units
            t_p1, minp1_i, redp1_i = fat_unit(rb_p[1], r_p[1], wpb2[1], cK2[1], W63p[1])
            tile.add_dep_helper(minp1_i.ins, clip0_i.ins, sync=False,
                                reason="finish half-0 before unit 3 so action-half-0 matmul runs early")
            t_m1, minm1_i, redm1_i = fat_unit(rb_m[1], r_m[1], wmb2[1], cKm2[1], W63m[1])
            finish_half(1, t_p1, t_m1)
            pe_warm(redp1_i)
            nc.sync.dma_start(out=d_actp[:], in_=actp[:])
            nc.tensor.matmul(ns_ps[:], actT[1], wns(9), start=False, stop=True)

            ns_nat = wpool.tile([BL, S], DT, tag="ns_nat")
            nc.vector.tensor_copy(ns_nat[:], ns_ps[:])
            nc.sync.dma_start(out=d_ns[:], in_=ns_nat[:])

            # ---- transpose new_state back to [s, b] for the out matmuls --
            nsT, ns2T = [], []
            for k in range(4):
                ps = ppool.tile([128, BL], DT, tag="ps")
                nc.tensor.transpose(ps[:], ns_nat[:, k * 128:(k + 1) * 128], ident[:])
                t = wpool.tile([128, BL], DT, tag=f"nsT{k}")
                nc.vector.tensor_copy(t[:], ps[:])
                nsT.append(t)
                t2 = wpool.tile([128, BL], DT, tag=f"ns2T{k}")
                nc.vector.tensor_tensor(t2[:], t[:], t[:], ALU.mult)
                ns2T.append(t2)

            # ---- frequency = ns @ select_w ; loss0 = ns^2 @ saw ----------
            fq_ps = ppool.tile([BL, A], DT, tag="ps")
            for k in range(4):
                nc.tensor.matmul(fq_ps[:], nsT[k][:], selwk(k),
                                 start=(k == 0), stop=(k == 3))
            fql = wpool.tile([BL, A + 1], DT, tag="fql")
            nc.vector.tensor_copy(fql[:, 0:A], fq_ps[:])

            ls_ps = ppool.tile([BL, 1], DT, tag="ps")
            for k in range(4):
                nc.tensor.matmul(ls_ps[:], ns2T[k][:], sawma[:, k:k + 1],
                                 start=(k == 0), stop=(k == 3))
            nc.vector.tensor_copy(fql[:, A:A + 1], ls_ps[:])
            nc.sync.dma_start(out=d_fql[:], in_=fql[:])

    nc.compile()
    return nc


_NC_CACHE = None


def _get_nc():
    global _NC_CACHE
    if _NC_CACHE is None:
        _NC_CACHE = build_nc()
    return _NC_CACHE


def _pack_rows(a, width):
    """[R, C] with R = 128*n  ->  [128, n*C] panel (blocks along free dim)."""
    r, c = a.shape
    n = r // 128
    return np.ascontiguousarray(
        a.reshape(n, 128, c).transpose(1, 0, 2).reshape(128, n * c))


def prepare_in_maps(inputs):
    x = {k: np.asarray(v) for k, v in inputs.items()}
    f32 = lambda a: np.ascontiguousarray(a, dtype=np.float32)

    wns = np.concatenate([x["state_transfer1"], x["state_transfer2"],
                          x["state_transfer3_Pm"], x["state_transfer4"]], axis=0)
    sawma = np.zeros((128, 5), np.float32)
    sawma[:, 0:4] = x["select_add_w"].reshape(4, 128).T
    sawma[:, 4] = np.float32(x["max_action"])

    shared = {
        "pk": f32(_pack_rows(np.concatenate(
            [x["w_plus_temp0"], x["b_plus_temp0"],
             x["w_minus_temp0"], x["b_minus_temp0"]], axis=1), 4 * I)),
        "selwp": f32(_pack_rows(x["select_w"], A)),
        "seldp": f32(_pack_rows(x["select_delta"], A)),
        "sawma": sawma,
        "stFTp": f32(_pack_rows(x["state_transferF"].T, A)),
        "st3": f32(x["state_transfer3"][None, :]),
        "wnsa": f32(_pack_rows(wns[:640], S)),
        "wnsb": f32(_pack_rows(wns[640:], S)),
    }
    in_maps = []
    for k in range(NCORES):
        sl = slice(k * BL, (k + 1) * BL)
        m = dict(shared)
        m["prevp"] = f32(_pack_rows(x["prev_output"][sl].T, BL))
        m["inpp"] = f32(_pack_rows(x["inputs"][sl].T, BL))
        in_maps.append(m)
    return in_maps


def postprocess(res):
    loss0 = np.concatenate([res[k]["fql_out"][:, A:A + 1] for k in range(NCORES)], axis=0)
    frequency = np.concatenate([res[k]["fql_out"][:, 0:A] for k in range(NCORES)], axis=0)
    action = np.concatenate(
        [np.concatenate([res[k]["actp_out"][:, 0:BL],
                         res[k]["actp_out"][:, BL:2 * BL]], axis=0).T
         for k in range(NCORES)], axis=0)
    new_state = np.concatenate([res[k]["ns_out"] for k in range(NCORES)], axis=0)
    return (loss0, frequency, action, new_state)


def kernel(**inputs):
    in_maps = prepare_in_maps(inputs)
    res = run_bass_kernel_spmd(_get_nc(), in_maps, list(range(NCORES))).results
    return postprocess(res)


# revision 23
# speedup vs baseline: 1.2930x; 1.0142x over previous
"""Trainium2 Bass kernel for nn_CustomRNNCell (Kuramoto-style RNN cell).

Strategy: pure data parallelism over the batch dim (B=512 -> 64 rows/core,
8 cores), parameters replicated.  All activations live on-chip in a
"transposed" layout [feature, batch] so every weight matrix is consumed by
the PE untransposed; the host does the input transposes / output
un-transposes / tensor packing (pure data movement).

Key algebraic restructurings (validated against the reference):
  * w_recover / b_recover are difference / strict-cumsum operators, so the
    basis-parameter prep is a square + shifted subtract + prefix scan on the
    DVE (no matmuls), and W63 = sum_i w_i is just the last column of wp^2.
  * delta_term = sin(d)*(F @ cos(d)) - cos(d)*(F @ sin(d))   (angle-difference
    expansion; kills the [B,A,A] sin grid)
  * piecewise-linear basis: with c = cumsum(b_t0^2) (knots, increasing),
      sum_i w_i*relu(f - c_i) = W63*relu(f) - sum_i w_i*min(c_i, relu(f))
    (the sum-w*c constants cancel between the two relu branches), computed
    with broadcast access patterns on the DVE; the fat min/mult/fold passes
    only ever see values <= c_max ~ 0.6 so they run in bf16, while the
    dominant W63*relu(f) term stays fp32.
  * clip(x,-m,m) = min(max(x,-m),m) as one tensor_scalar op.
  * new_state via one PSUM-accumulated matmul over the stacked
    [1; prev; inputs; delta_term; action] x [st3; st1; st3_Pm; st2; st4];
    everything not needing `action` accumulates while the basis runs.
  * params are host-packed into a handful of [128, N] panels -> one DMA each
    (the HWDGE ring serializes DMAs at ~0.6us apiece), split across the two
    HWDGE rings (sync + scalar).
"""

import sys

for _p in ("/opt/trn_rl_repo",):
    if _p not in sys.path:
        sys.path.insert(0, _p)

import numpy as np

import concourse.bacc as bacc
import concourse.mybir as mybir
import concourse.tile as tile
from concourse.bass_utils import run_bass_kernel_spmd
from concourse.masks import make_identity

B, A, I = 512, 256, 64
S, P = 512, 256
NCORES = 8
BL = B // NCORES  # 64 batch rows per core

DT = mybir.dt.float32
BF = mybir.dt.bfloat16
AX = mybir.AxisListType
ALU = mybir.AluOpType
ACTF = mybir.ActivationFunctionType

PI = float(np.pi)


def build_nc():
    nc = bacc.Bacc()

    # ---- DRAM I/O (host-packed panels) -------------------------------
    # pk: [128, (half, param, i)] = natural-layout wp_t0|bp_t0|wm_t0|bm_t0
    d_pk = nc.dram_tensor("pk", [128, 2 * 4 * I], DT, kind="ExternalInput")
    d_prevp = nc.dram_tensor("prevp", [128, 4 * BL], DT, kind="ExternalInput")
    d_selw = nc.dram_tensor("selwp", [128, 4 * A], DT, kind="ExternalInput")
    d_seld = nc.dram_tensor("seldp", [128, 4 * A], DT, kind="ExternalInput")
    d_sawma = nc.dram_tensor("sawma", [128, 5], DT, kind="ExternalInput")
    d_stFTp = nc.dram_tensor("stFTp", [128, 2 * A], DT, kind="ExternalInput")
    d_inpp = nc.dram_tensor("inpp", [128, 2 * BL], DT, kind="ExternalInput")
    d_st3 = nc.dram_tensor("st3", [1, S], DT, kind="ExternalInput")
    d_wnsa = nc.dram_tensor("wnsa", [128, 5 * S], DT, kind="ExternalInput")
    d_wnsb = nc.dram_tensor("wnsb", [128, 5 * S], DT, kind="ExternalInput")

    d_ns = nc.dram_tensor("ns_out", [BL, S], DT, kind="ExternalOutput")
    d_fql = nc.dram_tensor("fql_out", [BL, A + 1], DT, kind="ExternalOutput")
    d_actp = nc.dram_tensor("actp_out", [128, 2 * BL], DT, kind="ExternalOutput")

    with tile.TileContext(nc) as tc:
        with (
            tc.tile_pool(name="const", bufs=1) as cpool,
            tc.tile_pool(name="work", bufs=2) as wpool,
            tc.tile_pool(name="fat", bufs=3) as fatpool,
            tc.tile_pool(name="psum", bufs=6, space="PSUM") as ppool,
            tc.tile_pool(name="psum_ns", bufs=1, space="PSUM") as ppool_ns,
        ):
            # ---- input DMAs: one per panel, in order of need -----------
            pk = cpool.tile([128, 2 * 4 * I], DT, tag="pk")
            nc.sync.dma_start(out=pk[:], in_=d_pk[:])
            selw = cpool.tile([128, 4 * A], DT, tag="selw")
            nc.sync.dma_start(out=selw[:], in_=d_selw[:])
            prevp = cpool.tile([128, 4 * BL], DT, tag="prevp")
            nc.sync.dma_start(out=prevp[:], in_=d_prevp[:])
            # the rest goes on the ACT HWDGE ring, in parallel
            sawma = cpool.tile([128, 5], DT, tag="sawma")
            nc.scalar.dma_start(out=sawma[:], in_=d_sawma[:])
            seld = cpool.tile([128, 4 * A], DT, tag="seld")
            nc.scalar.dma_start(out=seld[:], in_=d_seld[:])
            stFTp = cpool.tile([128, 2 * A], DT, tag="stFTp")
            nc.scalar.dma_start(out=stFTp[:], in_=d_stFTp[:])
            inpp = cpool.tile([128, 2 * BL], DT, tag="inpp")
            nc.scalar.dma_start(out=inpp[:], in_=d_inpp[:])
            st3 = cpool.tile([1, S], DT, tag="st3")
            nc.scalar.dma_start(out=st3[:], in_=d_st3[:])
            wnsa = cpool.tile([128, 5 * S], DT, tag="wnsa")
            nc.scalar.dma_start(out=wnsa[:], in_=d_wnsa[:])
            wnsb = cpool.tile([128, 5 * S], DT, tag="wnsb")
            nc.scalar.dma_start(out=wnsb[:], in_=d_wnsb[:])

            def prevT(k):
                return prevp[:, k * BL:(k + 1) * BL]

            def selwk(k):  # [128, 256] K-tile of select_w
                return selw[:, k * A:(k + 1) * A]

            def seldk(k):
                return seld[:, k * A:(k + 1) * A]

            def wns(k):
                if k < 5:
                    return wnsa[:, k * S:(k + 1) * S]
                return wnsb[:, (k - 5) * S:(k - 5 + 1) * S]

            ident = cpool.tile([BL, BL], DT, tag="ident")
            make_identity(nc, ident[:])
            ones_row = cpool.tile([1, BL], DT, tag="ones_row")
            nc.vector.memset(ones_row[:], 1.0)
            ma_col = sawma[:, 4:5]
            bias_hpi = cpool.tile([128, 1], DT, tag="bias_hpi")
            nc.vector.memset(bias_hpi[:], PI / 2)

            # ---- param prep, all on the DVE ------------------------------
            # sq2 = pk^2 ; per (half, param) slices are [128, 64]
            sq2 = wpool.tile([128, 2 * 4 * I], DT, tag="sq2")
            nc.vector.tensor_tensor(sq2[:], pk[:], pk[:], ALU.mult)

            def sqs(half, j):
                o = half * 4 * I + j * I
                return sq2[:, o:o + I], sq2[:, o + I - 1:o + I]  # slice, last col

            # w_plus = diff(wp2) (bf16), "wmb" = +diff(wm2) = -w_minus (bf16),
            # c = exclusive-cumsum(bp2), c' = exclusive-cumsum(bm2) (fp32)
            wpb, wmb, cK, cKm, W63p, W63m = [], [], [], [], [], []

            def prep_half(half):
                wp2, wp2last = sqs(half, 0)
                bp2, _ = sqs(half, 1)
                wm2, wm2last = sqs(half, 2)
                bm2, _ = sqs(half, 3)
                W63p.append(wp2last)
                W63m.append(wm2last)  # positive; sign handled in finish_half

                t = wpool.tile([128, I], BF, tag=f"wpb{half}")
                nc.vector.tensor_copy(t[:, 0:1], wp2[:, 0:1])
                nc.vector.tensor_tensor(t[:, 1:I], wp2[:, 1:I], wp2[:, 0:I - 1],
                                        ALU.subtract)
                wpb.append(t)
                t = wpool.tile([128, I], BF, tag=f"wmb{half}")
                nc.vector.tensor_copy(t[:, 0:1], wm2[:, 0:1])
                nc.vector.tensor_tensor(t[:, 1:I], wm2[:, 1:I], wm2[:, 0:I - 1],
                                        ALU.subtract)
                wmb.append(t)
                for src2, lst in ((bp2, cK), (bm2, cKm)):
                    s = wpool.tile([128, I], DT, tag="scan")
                    nc.vector.tensor_tensor_scan(s[:], src2, src2, 0.0,
                                                 ALU.add, ALU.bypass)
                    cc = wpool.tile([128, I], DT, tag=f"c{len(lst)}_{half}")
                    nc.vector.tensor_tensor(cc[:], s[:], src2, ALU.subtract)
                    lst.append(cc)
                # x2-expanded bf16 copies for the paired-innermost fat passes
                for nm2, (src_t, lst) in (("wp", (wpb[half], wpb2)),
                                          ("wm", (wmb[half], wmb2)),
                                          ("cp", (cK[half], cK2)),
                                          ("cm", (cKm[half], cKm2))):
                    e = wpool.tile([128, 2 * I], BF, tag=f"x2_{nm2}_{half}")
                    nc.vector.tensor_copy(
                        e[:].rearrange("p (i t) -> p i t", t=2),
                        src_t[:].rearrange("p (i u) -> p i u", u=1)
                        .broadcast_to([128, I, 2]))
                    lst.append(e)

            wpb2, wmb2, cK2, cKm2 = [], [], [], []
            prep_half(0)

            # ---- freq / delta:  fdT = sel^T-as-lhsT @ prevT --------------
            fd_ps = []
            for m in range(4):
                ps = ppool.tile([128, BL], DT, tag="ps")
                for k in range(4):
                    lhs = (selwk(k) if m < 2 else seldk(k))
                    mm = m % 2
                    nc.tensor.matmul(ps[:], lhs[:, mm * 128:(mm + 1) * 128],
                                     prevT(k), start=(k == 0), stop=(k == 3))
                fd_ps.append(ps)

            # r+ = relu(f), r- = relu(-f)  (fp32, straight from PSUM)
            r_p, r_m, rb_p, rb_m = [], [], [], []

            def r_half(half):
                rp = wpool.tile([128, BL], DT, tag=f"r_p{half}")
                nc.vector.tensor_scalar(rp[:], fd_ps[half][:], 0.0, None, ALU.max)
                rm = wpool.tile([128, BL], DT, tag=f"r_m{half}")
                nc.vector.tensor_scalar(rm[:], fd_ps[half][:], -1.0, 0.0,
                                        ALU.mult, ALU.max)
                r_p.append(rp)
                r_m.append(rm)
                rbp = wpool.tile([128, BL], BF, tag=f"rb_p{half}")
                nc.vector.tensor_copy(rbp[:], rp[:])
                rbm = wpool.tile([128, BL], BF, tag=f"rb_m{half}")
                nc.vector.tensor_copy(rbm[:], rm[:])
                rb_p.append(rbp)
                rb_m.append(rbm)

            r_half(0)

            # ---- basis fat-pass helper ----------------------------------
            # Fat layout [p][b_hi=32][i=64][b_lo=2]: every operand gets a
            # dense step-1 innermost pair, which keeps the DVE in 2x mode
            # (a plain [b][i] layout leaves one operand with a stride-0
            # innermost -> 1x).  b = 2*b_hi + b_lo stays in natural order.
            def fat_unit(rb, r, w2, c2, w63):
                r_bc = (rb[:].rearrange("p (bh bl) -> p bh bl", bl=2)
                        .rearrange("p bh (u bl) -> p bh u bl", u=1)
                        .broadcast_to([128, BL // 2, I, 2]))
                c_bc = (c2[:].rearrange("p (i bl) -> p i bl", bl=2)
                        .rearrange("p (u i) bl -> p u i bl", u=1)
                        .broadcast_to([128, BL // 2, I, 2]))
                w_bc = (w2[:].rearrange("p (i bl) -> p i bl", bl=2)
                        .rearrange("p (u i) bl -> p u i bl", u=1)
                        .broadcast_to([128, BL // 2, I, 2]))
                M = fatpool.tile([128, BL * I], BF, tag="M")
                M4 = M[:].rearrange("p (bh i bl) -> p bh i bl", i=I, bl=2)
                min_inst = nc.vector.tensor_tensor(M4, r_bc, c_bc, ALU.min)
                Y = fatpool.tile([128, BL * I], BF, tag="Y")
                Y4 = Y[:].rearrange("p (bh i bl) -> p bh i bl", i=I, bl=2)
                nc.vector.tensor_tensor(Y4, M4, w_bc, ALU.mult)
                cur, cur_i = Y4, I
                while cur_i > 2:
                    ni = cur_i // 2
                    F = fatpool.tile([128, BL * ni], BF, tag=f"F{ni}")
                    F4 = F[:].rearrange("p (bh i bl) -> p bh i bl", i=ni, bl=2)
                    nc.vector.tensor_tensor(F4, cur[:, :, 0:ni, :],
                                            cur[:, :, ni:cur_i, :], ALU.add)
                    cur, cur_i = F4, ni
                red = wpool.tile([128, BL], DT, tag="red")
                red4 = red[:].rearrange("p (bh i bl) -> p bh i bl", i=1, bl=2)
                red_inst = nc.vector.tensor_tensor(red4, cur[:, :, 0:1, :],
                                                   cur[:, :, 1:2, :], ALU.add)
                t = wpool.tile([128, BL], DT, tag="tbr")
                nc.vector.scalar_tensor_tensor(t[:], r[:], w63, red[:],
                                               ALU.mult, ALU.subtract)
                return t, min_inst, red_inst

            actp = wpool.tile([128, 2 * BL], DT, tag="actp")
            actT = []

            def pe_warm(gate_inst):
                # tiny matmul gated on basis progress so the PE's HAM clock
                # never re-throttles before the tail matmuls
                psd = ppool.tile([1, 16], DT, tag="ps")
                mm = nc.tensor.matmul(psd[:], ones_row[:, 0:1], st3[:, 0:16],
                                      start=True, stop=True)
                tile.add_dep_helper(mm.ins, gate_inst.ins, sync=True,
                                    reason="HAM warm-keeper")

            def finish_half(half, tp, tm):
                # anc = tp - tm  (tm carries +diff weights = -w_minus terms)
                anc = wpool.tile([128, BL], DT, tag="anc")
                nc.vector.tensor_tensor(anc[:], tp[:], tm[:], ALU.subtract)
                at = actp[:, half * BL:(half + 1) * BL]
                clip_inst = nc.vector.tensor_scalar(at, anc[:], nma_col[:], ma_col,
                                                    ALU.max, ALU.min)
                actT.append(at)
                return clip_inst

            # FAT unit (+,0) goes first so the DVE gets busy asap
            t_p0, minp0_i, redp0_i = fat_unit(rb_p[0], r_p[0], wpb2[0], cK2[0], W63p[0])
            r_half(1)
            prep_half(1)
            nma_col = cpool.tile([128, 1], DT, tag="nma")
            nc.vector.tensor_scalar(nma_col[:], ma_col, -1.0, None, ALU.mult)

            # ---- sin / cos of delta (range-reduced; fills DVE/ACT gaps) --
            # y = x - 2pi*k via an int32 cast (round-to-nearest on HW,
            # trunc in CoreSim); a branch-free +-2pi correction makes the
            # result [-pi, pi] under either conversion mode.
            sinT, cosT = [], []
            for half in range(2):
                ki = wpool.tile([128, BL], mybir.dt.int32, tag="sc_ki")
                nc.vector.tensor_scalar(ki[:], fd_ps[2 + half][:],
                                        float(1 / (2 * PI)), 32.0, ALU.mult, ALU.add)
                xoff = wpool.tile([128, BL], DT, tag="sc_xoff")
                nc.vector.tensor_scalar(xoff[:], fd_ps[2 + half][:], float(64 * PI),
                                        None, ALU.add)
                y1 = wpool.tile([128, BL], DT, tag="sc_y1")
                nc.vector.scalar_tensor_tensor(y1[:], ki[:], float(-2 * PI), xoff[:],
                                               ALU.mult, ALU.add)
                w = wpool.tile([128, BL], DT, tag="sc_w")
                nc.vector.tensor_scalar(w[:], y1[:], PI, float(-2 * PI),
                                        ALU.is_gt, ALU.mult)
                y2 = wpool.tile([128, BL], DT, tag="sc_y2")
                nc.vector.tensor_tensor(y2[:], y1[:], w[:], ALU.add)
                y = wpool.tile([128, BL], DT, tag="sc_y")
                nc.vector.tensor_scalar(y[:], y2[:], -PI, PI, ALU.max, ALU.min)
                s = wpool.tile([128, BL], DT, tag=f"sinT{half}")
                nc.scalar.activation(s[:], y[:], ACTF.Sin)
                # cos(y) = sin(pi/2 - |y|),  argument stays in [-pi/2, pi/2]
                ay = wpool.tile([128, BL], DT, tag="sc_ay")
                nc.scalar.activation(ay[:], y[:], ACTF.Abs)
                c = wpool.tile([128, BL], DT, tag=f"cosT{half}")
                nc.scalar.activation(c[:], ay[:], ACTF.Sin, bias=bias_hpi[:],
                                     scale=-1.0)
                sinT.append(s)
                cosT.append(c)

            t_m0, minm0_i, redm0_i = fat_unit(rb_m[0], r_m[0], wmb2[0], cKm2[0], W63m[0])
            clip0_i = finish_half(0, t_p0, t_m0)
            pe_warm(redp0_i)
            pe_warm(redm0_i)

            # ---- U = F @ cos, V = F @ sin ; dtT = sin*U - cos*V ----------
            dtT = []
            for m in range(2):
                psU = ppool.tile([128, BL], DT, tag="ps")
                psV = ppool.tile([128, BL], DT, tag="ps")
                for k in range(2):
                    lhs = stFTp[:, k * A + m * 128:k * A + (m + 1) * 128]
                    nc.tensor.matmul(psU[:], lhs, cosT[k][:], start=(k == 0), stop=(k == 1))
                    uv_last = nc.tensor.matmul(psV[:], lhs, sinT[k][:],
                                               start=(k == 0), stop=(k == 1))
                t1 = wpool.tile([128, BL], DT, tag="dt_t1")
                nc.vector.tensor_tensor(t1[:], sinT[m][:], psU[:], ALU.mult)
                t2 = wpool.tile([128, BL], DT, tag="dt_t2")
                nc.vector.tensor_tensor(t2[:], cosT[m][:], psV[:], ALU.mult)
                t = wpool.tile([128, BL], DT, tag=f"dtT{m}")
                nc.vector.tensor_tensor(t[:], t1[:], t2[:], ALU.subtract)
                dtT.append(t)

            # ---- new_state stacked matmul: everything that doesn't need
            # action accumulates into PSUM while the basis runs -------------
            ns_ps = ppool_ns.tile([BL, S], DT, tag="ns_ps")
            ns_first = nc.tensor.matmul(ns_ps[:], ones_row[:], st3[:],
                                        start=True, stop=False)
            tile.add_dep_helper(ns_first.ins, uv_last.ins, sync=False,
                                reason="keep PE free for U/V before the wns stack")
            for k in range(4):
                nc.tensor.matmul(ns_ps[:], prevT(k), wns(k), start=False, stop=False)
            for k in range(2):
                nc.tensor.matmul(ns_ps[:], inpp[:, k * BL:(k + 1) * BL], wns(6 + k),
                                 start=False, stop=False)
            for k in range(2):
                nc.tensor.matmul(ns_ps[:], dtT[k][:], wns(4 + k),
                                 start=False, stop=False)
            # action half 0 as soon as it exists (wns block 8 = st4 rows 0:128)
            nc.tensor.matmul(ns_ps[:], actT[0], wns(8), start=False, stop=False)

            # remaining fat units
            t_p1, minp1_i, redp1_i = fat_unit(rb_p[1], r_p[1], wpb2[1], cK2[1], W63p[1])
            tile.add_dep_helper(minp1_i.ins, clip0_i.ins, sync=False,
                                reason="finish half-0 before unit 3 so action-half-0 matmul runs early")
            t_m1, minm1_i, redm1_i = fat_unit(rb_m[1], r_m[1], wmb2[1], cKm2[1], W63m[1])
            finish_half(1, t_p1, t_m1)
            pe_warm(redp1_i)
            nc.sync.dma_start(out=d_actp[:], in_=actp[:])
            nc.tensor.matmul(ns_ps[:], actT[1], wns(9), start=False, stop=True)

            ns_nat = wpool.tile([BL, S], DT, tag="ns_nat")
            nc.vector.tensor_copy(ns_nat[:], ns_ps[:])
            nc.sync.dma_start(out=d_ns[:], in_=ns_nat[:])

            # ---- transpose new_state back to [s, b] for the out matmuls --
            nsT, ns2T = [], []
            for k in range(4):
                ps = ppool.tile([128, BL], DT, tag="ps")
                nc.tensor.transpose(ps[:], ns_nat[:, k * 128:(k + 1) * 128], ident[:])
                t = wpool.tile([128, BL], DT, tag=f"nsT{k}")
                nc.vector.tensor_copy(t[:], ps[:])
                nsT.append(t)
                t2 = wpool.tile([128, BL], DT, tag=f"ns2T{k}")
                nc.vector.tensor_tensor(t2[:], t[:], t[:], ALU.mult)
                ns2T.append(t2)

            # ---- frequency = ns @ select_w ; loss0 = ns^2 @ saw ----------
            fq_ps = ppool.tile([BL, A], DT, tag="ps")
            for k in range(4):
                nc.tensor.matmul(fq_ps[:], nsT[k][:], selwk(k),
                                 start=(k == 0), stop=(k == 3))
            fql = wpool.tile([BL, A + 1], DT, tag="fql")
            nc.vector.tensor_copy(fql[:, 0:A], fq_ps[:])

            ls_ps = ppool.tile([BL, 1], DT, tag="ps")
            for k in range(4):
                nc.tensor.matmul(ls_ps[:], ns2T[k][:], sawma[:, k:k + 1],
                                 start=(k == 0), stop=(k == 3))
            nc.vector.tensor_copy(fql[:, A:A + 1], ls_ps[:])
            nc.sync.dma_start(out=d_fql[:], in_=fql[:])

    nc.compile()
    return nc


_NC_CACHE = None


def _get_nc():
    global _NC_CACHE
    if _NC_CACHE is None:
        _NC_CACHE = build_nc()
    return _NC_CACHE


def _pack_rows(a, width):
    """[R, C] with R = 128*n  ->  [128, n*C] panel (blocks along free dim)."""
    r, c = a.shape
    n = r // 128
    return np.ascontiguousarray(
        a.reshape(n, 128, c).transpose(1, 0, 2).reshape(128, n * c))


def prepare_in_maps(inputs):
    x = {k: np.asarray(v) for k, v in inputs.items()}
    f32 = lambda a: np.ascontiguousarray(a, dtype=np.float32)

    wns = np.concatenate([x["state_transfer1"], x["state_transfer2"],
                          x["state_transfer3_Pm"], x["state_transfer4"]], axis=0)
    sawma = np.zeros((128, 5), np.float32)
    sawma[:, 0:4] = x["select_add_w"].reshape(4, 128).T
    sawma[:, 4] = np.float32(x["max_action"])

    shared = {
        "pk": f32(_pack_rows(np.concatenate(
            [x["w_plus_temp0"], x["b_plus_temp0"],
             x["w_minus_temp0"], x["b_minus_temp0"]], axis=1), 4 * I)),
        "selwp": f32(_pack_rows(x["select_w"], A)),
        "seldp": f32(_pack_rows(x["select_delta"], A)),
        "sawma": sawma,
        "stFTp": f32(_pack_rows(x["state_transferF"].T, A)),
        "st3": f32(x["state_transfer3"][None, :]),
        "wnsa": f32(_pack_rows(wns[:640], S)),
        "wnsb": f32(_pack_rows(wns[640:], S)),
    }
    in_maps = []
    for k in range(NCORES):
        sl = slice(k * BL, (k + 1) * BL)
        m = dict(shared)
        m["prevp"] = f32(_pack_rows(x["prev_output"][sl].T, BL))
        m["inpp"] = f32(_pack_rows(x["inputs"][sl].T, BL))
        in_maps.append(m)
    return in_maps


def postprocess(res):
    loss0 = np.concatenate([res[k]["fql_out"][:, A:A + 1] for k in range(NCORES)], axis=0)
    frequency = np.concatenate([res[k]["fql_out"][:, 0:A] for k in range(NCORES)], axis=0)
    action = np.concatenate(
        [np.concatenate([res[k]["actp_out"][:, 0:BL],
                         res[k]["actp_out"][:, BL:2 * BL]], axis=0).T
         for k in range(NCORES)], axis=0)
    new_state = np.concatenate([res[k]["ns_out"] for k in range(NCORES)], axis=0)
    return (loss0, frequency, action, new_state)


def kernel(**inputs):
    in_maps = prepare_in_maps(inputs)
    last_err = None
    for _attempt in range(3):
        try:
            res = run_bass_kernel_spmd(_get_nc(), in_maps,
                                       list(range(NCORES))).results
            return postprocess(res)
        except Exception as e:  # transient device-unrecoverable errors
            last_err = e
    raise last_err
